# revision 15
# baseline (speedup 1.0000x reference)
"""Distributed Trainium2 kernel for the 4-block GNN (nn_ActorGNN).

Strategy (edge-parallel, dst-sharded):
  - Pad N=100000 -> NP=100352 = 8 * 12544 nodes; core c owns nodes
    [c*12544, (c+1)*12544).  Node features live transposed in SBUF (H^T).
  - Algebra: the edge MLP  relu([x_src|x_dst|ea] @ We + be)  is split as
    relu(U[src] + V[dst] + ea@WeE + be) with U = x@WeS, V = x@WeD computed
    per node shard (cheap N-side matmuls).
  - U is AllGathered (bf16) so every core can gather arbitrary source rows;
    V/agg stay core-local because edges are assigned to the core that owns
    their destination.
  - Edges are grouped by destination window (128 nodes) and padded to a
    uniform 36 chunks x 128 edges per window so all 8 cores run the same
    static program.  Per chunk: one-hot matrices S / S^T are built on DVE
    from per-edge local-dst values; PE matmuls expand V, add ea@WeE, inject
    gathered U rows (identity matmul), and segment-reduce messages into a
    per-window PSUM accumulator.  scatter-mean = accumulate + multiply by
    1/max(cnt,1).
"""

import numpy as np
import ml_dtypes

BF16 = ml_dtypes.bfloat16

N = 100_000
E = 3_200_000
D = 128
ED = 16
NCORES = 8
NS = 12_544           # nodes per core
NP = NS * NCORES      # padded node count
WN = 128              # nodes per window
NWIN = NS // WN       # 98 windows per core
CHW = 36              # chunks (of 128 edges) per window, uniform
ES = NWIN * CHW * 128  # padded edge slots per core
GRP = 4               # chunks per relu/u-inject group
SGRP = 12             # chunks per S-slab generation call (divisible by GRP)


# ---------------------------------------------------------------------------
# host-side preparation
# ---------------------------------------------------------------------------

def _prep_edges(edge_index, edge_attr):
    """Distribute edges to cores/windows; build per-core slot arrays."""
    src = edge_index[0].astype(np.int64)
    dst = edge_index[1].astype(np.int64)

    cnt = np.bincount(dst, minlength=NP).astype(np.float32)
    invc_full = 1.0 / np.maximum(cnt, 1.0)

    core = dst // NS
    win = (dst % NS) // WN
    l = dst % WN

    per_core = []
    for c in range(NCORES):
        m = core == c
        s_c, w_c, l_c = src[m], win[m], l[m]
        order = np.argsort(w_c, kind="stable")
        s_c, w_c, l_c = s_c[order], w_c[order], l_c[order]
        ea_c = edge_attr[m][order]

        counts = np.bincount(w_c, minlength=NWIN)
        assert counts.max() <= CHW * 128, f"window overflow: {counts.max()}"
        starts = np.concatenate([[0], np.cumsum(counts)])

        # slot arrays, (chunk, partition) order inside each window
        srcg = np.zeros((128, NWIN * CHW), dtype=np.int32)
        lcol = np.full((128, NWIN * CHW), -1.0, dtype=np.float32)
        lrow = np.full((1, ES), -1.0, dtype=np.float32)
        eat = np.zeros((ED, ES), dtype=np.float32)

        for w in range(NWIN):
            k = counts[w]
            sl = slice(starts[w], starts[w + 1])
            j = np.arange(k)
            ch = w * CHW + j // 128
            p = j % 128
            srcg[p, ch] = s_c[sl]
            lcol[p, ch] = l_c[sl]
            pos = ch * 128 + p
            lrow[0, pos] = l_c[sl]
            eat[:, pos] = ea_c[sl].T

        per_core.append(
            dict(
                srcg=srcg,
                lcol=lcol.astype(BF16),
                lrow=lrow.astype(BF16),
                eat=eat.astype(BF16),
                invc=invc_full[c * NS:(c + 1) * NS].reshape(NWIN, WN).T.copy(),
            )
        )
    return per_core


def _prep_inputs(inputs):
    x = inputs["x"]
    xp = np.zeros((NP, D), dtype=np.float32)
    xp[:N] = x
    per_core_edges = _prep_edges(np.asarray(inputs["edge_index"]),
                                 np.asarray(inputs["edge_attr"]))

    iotar = np.broadcast_to(np.arange(128, dtype=np.float32), (128, 128))
    iotac = np.arange(128, dtype=np.float32).reshape(128, 1)
    ones1 = np.ones((1, 128), dtype=np.float32)

    blocks = []
    for i in range(1, 5):
        We = np.asarray(inputs[f"We{i}"], np.float32)
        be = np.asarray(inputs[f"be{i}"], np.float32)
        Wn = np.asarray(inputs[f"Wn{i}"], np.float32)
        bn = np.asarray(inputs[f"bn{i}"], np.float32)
        din = We.shape[0] - ED
        din //= 2
        dout = We.shape[1]
        WeS, WeD, WeE = We[:din], We[din:2 * din], We[2 * din:]
        # pad dout -> 128
        wesd = np.zeros((128, 256), np.float32)
        wesd[:din, :dout] = WeS
        wesd[:din, 128:128 + dout] = WeD
        berow = np.zeros((1, 256), np.float32)
        berow[0, 128:128 + dout] = be
        wee = np.zeros((ED, 128), np.float32)
        wee[:, :dout] = WeE
        wnt = np.zeros((128, dout), np.float32)
        wnt[:din] = Wn[:din]
        wnb = np.zeros((128, dout), np.float32)
        wnb[:dout] = Wn[din:]
        bncol = bn.reshape(dout, 1).astype(np.float32)
        blocks.append(dict(wesd=wesd, berow=berow.astype(BF16),
                           wee=wee.astype(BF16), wnt=wnt,
                           wnb=wnb.astype(BF16), bn=bncol))

    in_maps = []
    for c in range(NCORES):
        m = dict(
            xT=xp[c * NS:(c + 1) * NS].T.copy(),
            srcg=per_core_edges[c]["srcg"],
            lcol=per_core_edges[c]["lcol"],
            lrow=per_core_edges[c]["lrow"],
            eat=per_core_edges[c]["eat"],
            invc=per_core_edges[c]["invc"],
            iotar=iotar.astype(BF16),
            iotac=iotac.astype(BF16),
            ones1=ones1.astype(BF16),
        )
        for i, b in enumerate(blocks, 1):
            for k, v in b.items():
                m[f"{k}{i}"] = v
        in_maps.append(m)
    return in_maps


# ---------------------------------------------------------------------------
# numpy emulation of the device dataflow (for fast correctness checking)
# ---------------------------------------------------------------------------

def _emulate(in_maps):
    import jax

    f32 = np.float32
    outs = []
    HT = [m["xT"].astype(f32).copy() for m in in_maps]
    for i in range(1, 5):
        dout = 1 if i == 4 else D
        Us, Vs = [], []
        for c, m in enumerate(in_maps):
            wesd = m[f"wesd{i}"].astype(f32)
            uv = HT[c].T @ wesd  # [NS, 256]
            uv += np.ones((NS, 1), f32) @ m[f"berow{i}"].astype(f32)
            Us.append(uv[:, :128].astype(BF16))
            Vs.append(uv[:, 128:].astype(BF16))
        U_full = np.concatenate(Us, 0)  # bf16 allgather
        for c, m in enumerate(in_maps):
            V = Vs[c].astype(f32)
            # slot s = g*128 + p  (g = global chunk)
            src_s = m["srcg"].T.reshape(-1)
            l_s = m["lcol"].T.reshape(-1).astype(f32)
            valid = l_s >= 0
            win_s = np.arange(ES) // (CHW * 128)
            dst_s = (win_s * 128 + l_s.astype(np.int64).clip(0))
            u = U_full[src_s].astype(f32)
            v = np.where(valid[:, None], V[dst_s], 0.0)
            ew = m["eat"].T.astype(f32) @ m[f"wee{i}"].astype(f32)
            msg = np.maximum(u + v + ew, 0).astype(BF16).astype(f32)
            agg = np.array(jax.ops.segment_sum(
                msg[valid], dst_s[valid], num_segments=NS))
            agg *= m["invc"].T.reshape(-1)[:, None]
            aggT = agg.astype(BF16).astype(f32).T
            hT = m[f"wnt{i}"].astype(f32).T @ HT[c]
            hT += m[f"wnb{i}"].astype(f32).T @ aggT
            hT += m[f"bn{i}"].astype(f32)
            if i < 4:
                HT[c] = np.maximum(hT, 0)
            else:
                outs.append(1.0 / (1.0 + np.exp(-hT[0])))
    return np.concatenate(outs)[:N].reshape(N, 1).astype(np.float32)


# ---------------------------------------------------------------------------
# bass program
# ---------------------------------------------------------------------------

def _build():
    from concourse import bacc, bass, mybir, tile
    from concourse.masks import make_identity

    f32 = mybir.dt.float32
    bf16 = mybir.dt.bfloat16
    i32 = mybir.dt.int32

    nc = bacc.Bacc("TRN2", num_devices=NCORES)

    inp = {}
    for name, shape, dt in [
        ("xT", [128, NS], f32),
        ("srcg", [128, NWIN * CHW], i32),
        ("lcol", [128, NWIN * CHW], bf16),
        ("eat", [ED, ES], bf16),
        ("invc", [128, NWIN], f32),
        ("iotar", [128, 128], bf16),
        ("iotac", [128, 1], bf16),
        ("ones1", [1, 128], bf16),
    ]:
        inp[name] = nc.dram_tensor(name, shape, dt, kind="ExternalInput")
    for i in range(1, 5):
        dout = 1 if i == 4 else D
        for name, shape, dt in [
            (f"wesd{i}", [128, 256], f32),
            (f"berow{i}", [1, 256], bf16),
            (f"wee{i}", [ED, 128], bf16),
            (f"wnt{i}", [128, dout], f32),
            (f"wnb{i}", [128, dout], bf16),
            (f"bn{i}", [dout, 1], f32),
        ]:
            inp[name] = nc.dram_tensor(name, shape, dt, kind="ExternalInput")
    out_ext = nc.dram_tensor("out", [1, NS], f32, kind="ExternalOutput")

    with tile.TileContext(nc) as tc:
        with tc.tile_pool(name="res", bufs=1) as res, \
             tc.tile_pool(name="sb", bufs=2) as sb, \
             tc.tile_pool(name="pbig", bufs=2, space="PSUM") as pbig, \
             tc.tile_pool(name="psmall", bufs=2, space="PSUM") as psmall, \
             tc.tile_pool(name="dram", bufs=1, space="DRAM") as dram:

            # resident tensors
            hT = res.tile([128, NS], f32)          # node features, transposed
            vrm = res.tile([128, NWIN * 128], bf16)  # V row-major, tile t at cols t*128
            aggT = res.tile([128, NS], bf16)
            invc_sb = res.tile([128, NWIN], f32)
            iotar_sb = res.tile([128, 128], bf16)
            iotac_sb = res.tile([128, 1], bf16)
            ones_sb = res.tile([1, 128], bf16)
            ident = res.tile([128, 128], bf16)

            nc.sync.dma_start(hT[:], inp["xT"][:])
            nc.sync.dma_start(invc_sb[:], inp["invc"][:])
            nc.sync.dma_start(iotar_sb[:], inp["iotar"][:])
            nc.sync.dma_start(iotac_sb[:], inp["iotac"][:])
            nc.sync.dma_start(ones_sb[:], inp["ones1"][:])
            make_identity(nc, ident[:])

            u_bounce = dram.tile([NS, 128], bf16)

            for i in range(1, 5):
                u_full = dram.tile([NP, 128], bf16, addr_space="Shared",
                                   name=f"u_full{i}", tag=f"u_full{i}")
                dout = 1 if i == 4 else D
                wesd = sb.tile([128, 256], f32, tag="wesd")
                berow = sb.tile([1, 256], bf16, tag="berow")
                wee = sb.tile([ED, 128], bf16, tag="wee")
                wnt = sb.tile([128, dout], f32, tag="wnt")
                wnb = sb.tile([128, dout], bf16, tag="wnb")
                bncol = sb.tile([dout, 1], f32, tag="bncol")
                nc.sync.dma_start(wesd[:], inp[f"wesd{i}"][:])
                nc.sync.dma_start(berow[:], inp[f"berow{i}"][:])
                nc.sync.dma_start(wee[:], inp[f"wee{i}"][:])
                nc.sync.dma_start(wnt[:], inp[f"wnt{i}"][:])
                nc.sync.dma_start(wnb[:], inp[f"wnb{i}"][:])
                nc.sync.dma_start(bncol[:], inp[f"bn{i}"][:])

                # ---- node-side: U,V tiles (row-major) ----
                for t in range(NWIN):
                    puv = pbig.tile([128, 512], f32, tag="pbig")
                    nc.tensor.matmul(out=puv[:, :256],
                                     lhsT=hT[:, t * 128:(t + 1) * 128],
                                     rhs=wesd[:], start=True, stop=False)
                    nc.tensor.matmul(out=puv[:, :256], lhsT=ones_sb[:],
                                     rhs=berow[:], start=False, stop=True)
                    utile = sb.tile([128, 128], bf16, tag="utile")
                    nc.scalar.copy(utile[:], puv[:, :128])
                    nc.vector.tensor_copy(vrm[:, t * 128:(t + 1) * 128],
                                          puv[:, 128:256])
                    nc.sync.dma_start(u_bounce[t * 128:(t + 1) * 128, :],
                                      utile[:])

                nc.gpsimd.collective_compute(
                    "AllGather", mybir.AluOpType.bypass,
                    replica_groups=[list(range(NCORES))],
                    ins=[u_bounce.opt()], outs=[u_full.opt()],
                )

                # ---- edge phase ----
                for w in range(NWIN):
                    srcg_sl = sb.tile([128, CHW], i32, tag="srcg")
                    nc.sync.dma_start(srcg_sl[:],
                                      inp["srcg"][:, w * CHW:(w + 1) * CHW])
                    lcol_sl = sb.tile([128, CHW], bf16, tag="lcolw")
                    nc.sync.dma_start(lcol_sl[:],
                                      inp["lcol"][:, w * CHW:(w + 1) * CHW])
                    uslab = sb.tile([128, CHW * 128], bf16, tag="uslab")
                    nc.gpsimd.indirect_dma_start(
                        out=uslab[:],
                        out_offset=None,
                        in_=u_full[:],
                        in_offset=bass.IndirectOffsetOnAxis(
                            ap=srcg_sl[:], axis=0),
                    )
                    eat_sl = sb.tile([ED, CHW * 128], bf16, tag="eat")
                    nc.sync.dma_start(
                        eat_sl[:], inp["eat"][:, w * ES // NWIN:
                                              (w + 1) * ES // NWIN])

                    pw = psmall.tile([128, 128], f32, tag="pw")
                    for half in range(CHW // SGRP):
                        c0 = half * SGRP
                        sslab = sb.tile([128, SGRP * 128], bf16, tag="sslab")
                        lc3 = lcol_sl[:, c0:c0 + SGRP]
                        nc.vector.tensor_tensor(
                            out=sslab[:].rearrange("p (c e) -> p c e", c=SGRP),
                            in0=lc3[:, :, None].to_broadcast([128, SGRP, 128]),
                            in1=iotar_sb[:, None, :].to_broadcast(
                                [128, SGRP, 128]),
                            op=mybir.AluOpType.is_equal)

                        for g0 in range(c0, c0 + SGRP, GRP):
                            pst = psmall.tile([128, GRP * 128], bf16,
                                              tag="pst")
                            for c in range(g0, g0 + GRP):
                                r = (c - g0) * 128
                                cc = c - c0
                                nc.tensor.transpose(
                                    pst[:, r:r + 128],
                                    sslab[:, cc * 128:(cc + 1) * 128],
                                    ident[:])
                            stgrp = sb.tile([128, GRP * 128], bf16,
                                            tag="stgrp")
                            nc.any.tensor_copy(stgrp[:], pst[:])
                            pe_ = pbig.tile([128, 512], f32, tag="pbig")
                            for c in range(g0, g0 + GRP):
                                r = (c - g0) * 128
                                nc.tensor.matmul(
                                    out=pe_[:, r:r + 128],
                                    lhsT=stgrp[:, r:r + 128],
                                    rhs=vrm[:, w * 128:(w + 1) * 128],
                                    start=True, stop=False)
                                nc.tensor.matmul(
                                    out=pe_[:, r:r + 128],
                                    lhsT=eat_sl[:, c * 128:(c + 1) * 128],
                                    rhs=wee[:], start=False, stop=False)
                                nc.tensor.matmul(
                                    out=pe_[:, r:r + 128],
                                    lhsT=ident[:],
                                    rhs=uslab[:, c * 128:(c + 1) * 128],
                                    start=False, stop=True)
                            wslab = sb.tile([128, GRP * 128], bf16, tag="wslab")
                            nc.scalar.activation(
                                wslab[:], pe_[:],
                                mybir.ActivationFunctionType.Relu)
                            for c in range(g0, g0 + GRP):
                                r = (c - g0) * 128
                                cc = c - c0
                                nc.tensor.matmul(
                                    out=pw[:],
                                    lhsT=sslab[:, cc * 128:(cc + 1) * 128],
                                    rhs=wslab[:, r:r + 128],
                                    start=(c == 0), stop=(c == CHW - 1))
                    # scatter-mean + transpose into aggT
                    argm = sb.tile([128, 128], bf16, tag="argm")
                    nc.vector.tensor_scalar(
                        out=argm[:], in0=pw[:],
                        scalar1=invc_sb[:, w:w + 1], scalar2=None,
                        op0=mybir.AluOpType.mult)
                    pt = psmall.tile([128, 128], bf16, tag="pt")
                    nc.tensor.transpose(pt[:], argm[:], ident[:])
                    nc.scalar.copy(aggT[:, w * 128:(w + 1) * 128], pt[:])

                # ---- node update ----
                nsz = [512] * (NS // 512) + ([NS % 512] if NS % 512 else [])
                off = 0
                for sz in nsz:
                    ph = pbig.tile([128, 512], f32, tag="pbig")
                    nc.tensor.matmul(out=ph[:dout, :sz], lhsT=wnt[:],
                                     rhs=hT[:, off:off + sz],
                                     start=True, stop=False)
                    nc.tensor.matmul(out=ph[:dout, :sz], lhsT=wnb[:],
                                     rhs=aggT[:, off:off + sz],
                                     start=False, stop=True)
                    if i < 4:
                        nc.scalar.activation(
                            hT[:, off:off + sz], ph[:, :sz],
                            mybir.ActivationFunctionType.Relu,
                            bias=bncol[:])
                    else:
                        out_t = sb.tile([1, 512], f32, tag="out_t")
                        nc.scalar.activation(
                            out_t[:, :sz], ph[:dout, :sz],
                            mybir.ActivationFunctionType.Sigmoid,
                            bias=bncol[:])
                        nc.sync.dma_start(out_ext[:, off:off + sz],
                                          out_t[:, :sz])
                    off += sz

    nc.finalize()
    return nc


_NC_CACHE = {}


def kernel(**inputs):
    from concourse.bass_utils import run_bass_kernel_spmd

    in_maps = _prep_inputs(inputs)
    in_maps = [{k: v for k, v in m.items() if k != "lrow"} for m in in_maps]
    if "nc" not in _NC_CACHE:
        _NC_CACHE["nc"] = _build()
    nc = _NC_CACHE["nc"]
    res = run_bass_kernel_spmd(nc, in_maps, core_ids=list(range(NCORES)))
    outs = [res.results[c]["out"].reshape(-1) for c in range(NCORES)]
    return np.concatenate(outs)[:N].reshape(N, 1).astype(np.float32)


# revision 20
# speedup vs baseline: 2.4811x; 2.4811x over previous
"""Distributed Trainium2 kernel for the 4-block GNN (nn_ActorGNN).

Strategy (edge-parallel, dst-sharded):
  - Pad N=100000 -> NP=100352 = 8 * 12544 nodes; core c owns nodes
    [c*12544, (c+1)*12544).  Node features live transposed in SBUF (H^T).
  - Algebra: the edge MLP  relu([x_src|x_dst|ea] @ We + be)  is split as
    relu(U[src] + V[dst] + ea@WeE + be) with U = x@WeS, V = x@WeD computed
    per node shard (cheap N-side matmuls).
  - U is AllGathered (bf16); every core gathers arbitrary source rows with
    SWDGE indirect DMA; V/agg stay core-local (edges live on the core that
    owns their destination).
  - Edges are grouped by destination window of 112 nodes and padded to a
    uniform 32 chunks x 128 edges per window (SPMD-uniform).  112 was
    chosen so the expand matmul's contraction packs [S^T(112) ; ea(16)]
    against [V_win(112) ; WeE(16)] - the edge-attr matmul rides along for
    free.  Per chunk, PE does only two matmuls: the merged expand and the
    one-hot segment-reduce; gathered U rows are injected via one
    identity-matmul per 4 chunks.  scatter-mean = PSUM accumulate over the
    window + multiply by 1/max(cnt,1).
"""

import numpy as np
import ml_dtypes

BF16 = ml_dtypes.bfloat16

N = 100_000
E = 3_200_000
D = 128
ED = 16
NCORES = 8
NS = 12_544           # nodes per core
NP = NS * NCORES      # padded node count
WN = 112              # nodes per window (112 + 16 ea rows = 128 = PE K)
NWIN = NS // WN       # 112 windows per core
CHW = 32              # chunks (of 128 edges) per window, uniform
ES = NWIN * CHW * 128  # padded edge slots per core
GRP = 4               # chunks per relu/u-inject group
SGRP = 8              # chunks per S-slab generation call (divisible by GRP)


# ---------------------------------------------------------------------------
# host-side preparation
# ---------------------------------------------------------------------------

def _prep_edges(edge_index, edge_attr):
    """Distribute edges to cores/windows; build per-core slot arrays."""
    src = edge_index[0].astype(np.int64)
    dst = edge_index[1].astype(np.int64)

    cnt = np.bincount(dst, minlength=NP).astype(np.float32)
    invc_full = 1.0 / np.maximum(cnt, 1.0)

    core = dst // NS
    win = (dst % NS) // WN
    l = dst % WN

    per_core = []
    for c in range(NCORES):
        m = core == c
        s_c, w_c, l_c = src[m], win[m], l[m]
        order = np.argsort(w_c, kind="stable")
        s_c, w_c, l_c = s_c[order], w_c[order], l_c[order]
        ea_c = edge_attr[m][order]

        counts = np.bincount(w_c, minlength=NWIN)
        assert counts.max() <= CHW * 128, f"window overflow: {counts.max()}"
        starts = np.concatenate([[0], np.cumsum(counts)])

        # slot arrays, (chunk, partition) order inside each window
        srcg = np.zeros((128, NWIN * CHW), dtype=np.int32)
        lcol = np.full((128, NWIN * CHW), -1.0, dtype=np.float32)
        lrow = np.full((1, ES), -1.0, dtype=np.float32)
        eat = np.zeros((ED, ES), dtype=np.float32)

        for w in range(NWIN):
            k = counts[w]
            sl = slice(starts[w], starts[w + 1])
            j = np.arange(k)
            ch = w * CHW + j // 128
            p = j % 128
            srcg[p, ch] = s_c[sl]
            lcol[p, ch] = l_c[sl]
            pos = ch * 128 + p
            lrow[0, pos] = l_c[sl]
            eat[:, pos] = ea_c[sl].T

        per_core.append(
            dict(
                srcg=srcg,
                lcol=lcol.astype(BF16),
                lrow=lrow.astype(BF16),
                eat=eat.astype(BF16),
                invc=invc_full[c * NS:(c + 1) * NS].reshape(NWIN, WN).T.copy(),
            )
        )
    return per_core


def _prep_inputs(inputs):
    x = inputs["x"]
    xp = np.zeros((NP, D), dtype=np.float32)
    xp[:N] = x
    per_core_edges = _prep_edges(np.asarray(inputs["edge_index"]),
                                 np.asarray(inputs["edge_attr"]))

    iotar = np.broadcast_to(np.arange(WN, dtype=np.float32), (128, WN))
    iotac = np.arange(WN, dtype=np.float32).reshape(WN, 1)
    ones1 = np.ones((1, 128), dtype=np.float32)

    blocks = []
    for i in range(1, 5):
        We = np.asarray(inputs[f"We{i}"], np.float32)
        be = np.asarray(inputs[f"be{i}"], np.float32)
        Wn = np.asarray(inputs[f"Wn{i}"], np.float32)
        bn = np.asarray(inputs[f"bn{i}"], np.float32)
        din = We.shape[0] - ED
        din //= 2
        dout = We.shape[1]
        WeS, WeD, WeE = We[:din], We[din:2 * din], We[2 * din:]
        # pad dout -> 128
        wesd = np.zeros((128, 256), np.float32)
        wesd[:din, :dout] = WeS
        wesd[:din, 128:128 + dout] = WeD
        berow = np.zeros((1, 256), np.float32)
        berow[0, 128:128 + dout] = be
        wee = np.zeros((ED, 128), np.float32)
        wee[:, :dout] = WeE
        wnt = np.zeros((128, dout), np.float32)
        wnt[:din] = Wn[:din]
        wnb = np.zeros((128, dout), np.float32)
        wnb[:dout] = Wn[din:]
        bncol = bn.reshape(dout, 1).astype(np.float32)
        blocks.append(dict(wesd=wesd, berow=berow.astype(BF16),
                           wee=wee.astype(BF16), wnt=wnt,
                           wnb=wnb.astype(BF16), bn=bncol))

    in_maps = []
    for c in range(NCORES):
        m = dict(
            xT=xp[c * NS:(c + 1) * NS].T.copy(),
            srcg=per_core_edges[c]["srcg"],
            lcol=per_core_edges[c]["lcol"],
            lrow=per_core_edges[c]["lrow"],
            eat=per_core_edges[c]["eat"],
            invc=per_core_edges[c]["invc"],
            iotar=iotar.astype(BF16),
            iotac=iotac.astype(np.float32),
            ones1=ones1.astype(BF16),
        )
        for i, b in enumerate(blocks, 1):
            for k, v in b.items():
                m[f"{k}{i}"] = v
        in_maps.append(m)
    return in_maps


# ---------------------------------------------------------------------------
# numpy emulation of the device dataflow (for fast correctness checking)
# ---------------------------------------------------------------------------

def _emulate(in_maps):
    import jax

    f32 = np.float32
    outs = []
    HT = [m["xT"].astype(f32).copy() for m in in_maps]
    for i in range(1, 5):
        Us, Vs = [], []
        for c, m in enumerate(in_maps):
            wesd = m[f"wesd{i}"].astype(f32)
            uv = HT[c].T @ wesd  # [NS, 256]
            uv += np.ones((NS, 1), f32) @ m[f"berow{i}"].astype(f32)
            Us.append(uv[:, :128].astype(BF16))
            Vs.append(uv[:, 128:].astype(BF16))
        U_full = np.concatenate(Us, 0)  # bf16 allgather
        for c, m in enumerate(in_maps):
            V = Vs[c].astype(f32)
            # slot s = g*128 + p  (g = global chunk)
            src_s = m["srcg"].T.reshape(-1)
            l_s = m["lcol"].T.reshape(-1).astype(f32)
            valid = l_s >= 0
            win_s = np.arange(ES) // (CHW * 128)
            dst_s = (win_s * WN + l_s.astype(np.int64).clip(0))
            u = U_full[src_s].astype(f32)
            v = np.where(valid[:, None], V[dst_s], 0.0)
            ew = m["eat"].T.astype(f32) @ m[f"wee{i}"].astype(f32)
            msg = np.maximum(u + v + ew, 0).astype(BF16).astype(f32)
            agg = np.array(jax.ops.segment_sum(
                msg[valid], dst_s[valid], num_segments=NS))
            agg *= m["invc"].T.reshape(-1)[:, None]
            aggT = agg.astype(BF16).astype(f32).T
            hT = m[f"wnt{i}"].astype(f32).T @ HT[c]
            hT += m[f"wnb{i}"].astype(f32).T @ aggT
            hT += m[f"bn{i}"].astype(f32)
            if i < 4:
                HT[c] = np.maximum(hT, 0)
            else:
                outs.append(1.0 / (1.0 + np.exp(-hT[0])))
    return np.concatenate(outs)[:N].reshape(N, 1).astype(np.float32)


# ---------------------------------------------------------------------------
# bass program
# ---------------------------------------------------------------------------

def _build():
    from concourse import bacc, bass, mybir, tile
    from concourse.masks import make_identity

    f32 = mybir.dt.float32
    bf16 = mybir.dt.bfloat16
    i32 = mybir.dt.int32

    nc = bacc.Bacc("TRN2", num_devices=NCORES)

    inp = {}
    for name, shape, dt in [
        ("xT", [128, NS], f32),
        ("srcg", [128, NWIN * CHW], i32),
        ("lcol", [128, NWIN * CHW], bf16),
        ("lrow", [1, ES], bf16),
        ("eat", [ED, ES], bf16),
        ("invc", [WN, NWIN], f32),
        ("iotar", [128, WN], bf16),
        ("iotac", [WN, 1], f32),
        ("ones1", [1, 128], bf16),
    ]:
        inp[name] = nc.dram_tensor(name, shape, dt, kind="ExternalInput")
    for i in range(1, 5):
        dout = 1 if i == 4 else D
        for name, shape, dt in [
            (f"wesd{i}", [128, 256], f32),
            (f"berow{i}", [1, 256], bf16),
            (f"wee{i}", [ED, 128], bf16),
            (f"wnt{i}", [128, dout], f32),
            (f"wnb{i}", [128, dout], bf16),
            (f"bn{i}", [dout, 1], f32),
        ]:
            inp[name] = nc.dram_tensor(name, shape, dt, kind="ExternalInput")
    out_ext = nc.dram_tensor("out", [1, NS], f32, kind="ExternalOutput")

    with tile.TileContext(nc) as tc:
        with tc.tile_pool(name="res", bufs=1) as res, \
             tc.tile_pool(name="sb", bufs=2) as sb, \
             tc.tile_pool(name="pbig", bufs=2, space="PSUM") as pbig, \
             tc.tile_pool(name="psmall", bufs=2, space="PSUM") as psmall, \
             tc.tile_pool(name="dram", bufs=1, space="DRAM") as dram:

            # resident tensors
            hT = res.tile([128, NS], f32)          # node features, transposed
            # [V_win(112) ; WeE(16)] per window, window w at cols w*128
            vw = res.tile([128, NWIN * 128], bf16)
            aggT = res.tile([128, NS], bf16)
            invc_sb = res.tile([WN, NWIN], f32)
            iotar_sb = res.tile([128, WN], bf16)
            iotac_sb = res.tile([WN, 1], f32)
            ones_sb = res.tile([1, 128], bf16)
            ident = res.tile([128, 128], bf16)

            nc.sync.dma_start(hT[:], inp["xT"][:])
            nc.sync.dma_start(invc_sb[:], inp["invc"][:])
            nc.sync.dma_start(iotar_sb[:], inp["iotar"][:])
            nc.sync.dma_start(iotac_sb[:], inp["iotac"][:])
            nc.sync.dma_start(ones_sb[:], inp["ones1"][:])
            make_identity(nc, ident[:])

            u_bounce = dram.tile([NS, 128], bf16)

            for i in range(1, 5):
                dout = 1 if i == 4 else D
                u_full = dram.tile([NP, 128], bf16, addr_space="Shared",
                                   name=f"u_full{i}", tag=f"u_full{i}")
                wesd = sb.tile([128, 256], f32, tag="wesd")
                berow = sb.tile([1, 256], bf16, tag="berow")
                wnt = sb.tile([128, dout], f32, tag="wnt")
                wnb = sb.tile([128, dout], bf16, tag="wnb")
                bncol = sb.tile([dout, 1], f32, tag="bncol")
                nc.sync.dma_start(wesd[:], inp[f"wesd{i}"][:])
                nc.sync.dma_start(berow[:], inp[f"berow{i}"][:])
                nc.sync.dma_start(wnt[:], inp[f"wnt{i}"][:])
                nc.sync.dma_start(wnb[:], inp[f"wnb{i}"][:])
                nc.sync.dma_start(bncol[:], inp[f"bn{i}"][:])
                # WeE replicated into rows 112:128 of every window block
                nc.sync.dma_start(
                    vw[WN:128, :].rearrange("p (w d) -> p w d", w=NWIN),
                    inp[f"wee{i}"][:, None, :].to_broadcast([ED, NWIN, 128]))

                # ---- node-side: U,V tiles (row-major) ----
                for t in range(NWIN):
                    puv = pbig.tile([128, 512], f32, tag="pbig")
                    nc.tensor.matmul(out=puv[:WN, :256],
                                     lhsT=hT[:, t * WN:(t + 1) * WN],
                                     rhs=wesd[:], start=True, stop=False)
                    nc.tensor.matmul(out=puv[:WN, :256],
                                     lhsT=ones_sb[:, :WN],
                                     rhs=berow[:], start=False, stop=True)
                    utile = sb.tile([WN, 128], bf16, tag="utile")
                    nc.scalar.copy(utile[:], puv[:WN, :128])
                    nc.vector.tensor_copy(vw[:WN, t * 128:(t + 1) * 128],
                                          puv[:WN, 128:256])
                    nc.sync.dma_start(u_bounce[t * WN:(t + 1) * WN, :],
                                      utile[:])

                nc.gpsimd.collective_compute(
                    "AllGather", mybir.AluOpType.bypass,
                    replica_groups=[list(range(NCORES))],
                    ins=[u_bounce.opt()], outs=[u_full.opt()],
                )

                # ---- edge phase ----
                for w in range(NWIN):
                    srcg_sl = sb.tile([128, CHW], i32, tag="srcg")
                    nc.sync.dma_start(srcg_sl[:],
                                      inp["srcg"][:, w * CHW:(w + 1) * CHW])
                    lcol_sl = sb.tile([128, CHW], bf16, tag="lcolw")
                    nc.sync.dma_start(lcol_sl[:],
                                      inp["lcol"][:, w * CHW:(w + 1) * CHW])
                    uslab = sb.tile([128, CHW * 128], bf16, tag="uslab")
                    nc.gpsimd.indirect_dma_start(
                        out=uslab[:],
                        out_offset=None,
                        in_=u_full[:],
                        in_offset=bass.IndirectOffsetOnAxis(
                            ap=srcg_sl[:], axis=0),
                    )
                    # stacked stationary [S^T(112) ; ea(16)] for the window
                    stslab = sb.tile([128, CHW * 128], bf16, tag="stslab")
                    nc.sync.dma_start(
                        stslab[WN:128, :],
                        inp["eat"][:, w * CHW * 128:(w + 1) * CHW * 128])
                    lrow_bc = sb.tile([WN, CHW * 128], bf16, tag="lrowbc")
                    nc.sync.dma_start(
                        lrow_bc[:],
                        inp["lrow"][:, w * CHW * 128:(w + 1) * CHW * 128]
                        .to_broadcast([WN, CHW * 128]))
                    nc.vector.tensor_scalar(
                        out=stslab[:WN, :], in0=lrow_bc[:],
                        scalar1=iotac_sb[:], scalar2=None,
                        op0=mybir.AluOpType.is_equal)

                    pw = psmall.tile([128, 128], f32, tag="pw")
                    for half in range(CHW // SGRP):
                        c0 = half * SGRP
                        sslab = sb.tile([128, SGRP * WN], bf16, tag="sslab")
                        lc3 = lcol_sl[:, c0:c0 + SGRP]
                        nc.vector.tensor_tensor(
                            out=sslab[:].rearrange("p (c e) -> p c e", c=SGRP),
                            in0=lc3[:, :, None].to_broadcast([128, SGRP, WN]),
                            in1=iotar_sb[:, None, :].to_broadcast(
                                [128, SGRP, WN]),
                            op=mybir.AluOpType.is_equal)

                        for g0 in range(c0, c0 + SGRP, GRP):
                            pe_ = pbig.tile([128, 512], f32, tag="pbig")
                            nc.tensor.matmul(
                                out=pe_[:],
                                lhsT=ident[:],
                                rhs=uslab[:, g0 * 128:(g0 + GRP) * 128],
                                start=True, stop=False)
                            for c in range(g0, g0 + GRP):
                                r = (c - g0) * 128
                                nc.tensor.matmul(
                                    out=pe_[:, r:r + 128],
                                    lhsT=stslab[:, c * 128:(c + 1) * 128],
                                    rhs=vw[:, w * 128:(w + 1) * 128],
                                    start=False, stop=True)
                            wslab = sb.tile([128, GRP * 128], bf16, tag="wslab")
                            nc.scalar.activation(
                                wslab[:], pe_[:],
                                mybir.ActivationFunctionType.Relu)
                            for c in range(g0, g0 + GRP):
                                r = (c - g0) * 128
                                cc = c - c0
                                nc.tensor.matmul(
                                    out=pw[:WN, :],
                                    lhsT=sslab[:, cc * WN:(cc + 1) * WN],
                                    rhs=wslab[:, r:r + 128],
                                    start=(c == 0), stop=(c == CHW - 1))
                    # scatter-mean + transpose into aggT
                    argm = sb.tile([WN, 128], bf16, tag="argm")
                    nc.vector.tensor_scalar(
                        out=argm[:], in0=pw[:WN, :],
                        scalar1=invc_sb[:, w:w + 1], scalar2=None,
                        op0=mybir.AluOpType.mult)
                    pt = psmall.tile([128, WN], bf16, tag="pt")
                    nc.tensor.transpose(pt[:], argm[:], ident[:WN, :WN])
                    nc.scalar.copy(aggT[:, w * WN:(w + 1) * WN], pt[:])

                # ---- node update ----
                nsz = [512] * (NS // 512) + ([NS % 512] if NS % 512 else [])
                off = 0
                for sz in nsz:
                    ph = pbig.tile([128, 512], f32, tag="pbig")
                    nc.tensor.matmul(out=ph[:dout, :sz], lhsT=wnt[:],
                                     rhs=hT[:, off:off + sz],
                                     start=True, stop=False)
                    nc.tensor.matmul(out=ph[:dout, :sz], lhsT=wnb[:],
                                     rhs=aggT[:, off:off + sz],
                                     start=False, stop=True)
                    if i < 4:
                        nc.scalar.activation(
                            hT[:, off:off + sz], ph[:, :sz],
                            mybir.ActivationFunctionType.Relu,
                            bias=bncol[:])
                    else:
                        out_t = sb.tile([1, 512], f32, tag="out_t")
                        nc.scalar.activation(
                            out_t[:, :sz], ph[:dout, :sz],
                            mybir.ActivationFunctionType.Sigmoid,
                            bias=bncol[:])
                        nc.sync.dma_start(out_ext[:, off:off + sz],
                                          out_t[:, :sz])
                    off += sz

    nc.finalize()
    return nc


_NC_CACHE = {}


def kernel(**inputs):
    from concourse.bass_utils import run_bass_kernel_spmd

    in_maps = _prep_inputs(inputs)
    if "nc" not in _NC_CACHE:
        _NC_CACHE["nc"] = _build()
    nc = _NC_CACHE["nc"]
    res = run_bass_kernel_spmd(nc, in_maps, core_ids=list(range(NCORES)))
    outs = [res.results[c]["out"].reshape(-1) for c in range(NCORES)]
    return np.concatenate(outs)[:N].reshape(N, 1).astype(np.float32)


# revision 26
# speedup vs baseline: 2.8772x; 1.1597x over previous
"""Distributed Trainium2 kernel for the 4-block GNN (nn_ActorGNN).

Strategy (edge-parallel, dst-sharded):
  - Pad N=100000 -> NP=100352 = 8 * 12544 nodes; core c owns nodes
    [c*12544, (c+1)*12544).  Node features live transposed in SBUF (H^T).
  - Algebra: the edge MLP  relu([x_src|x_dst|ea] @ We + be)  is split as
    relu(U[src] + V[dst] + ea@WeE + be) with U = x@WeS, V = x@WeD computed
    per node shard (cheap N-side matmuls).
  - U is AllGathered (bf16); every core gathers arbitrary source rows with
    SWDGE indirect DMA; V/agg stay core-local (edges live on the core that
    owns their destination).
  - Edges are grouped by destination window of 112 nodes and padded to a
    uniform 32 chunks x 128 edges per window (SPMD-uniform).  112 was
    chosen so the expand matmul's contraction packs [S^T(112) ; ea(16)]
    against [V_win(112) ; WeE(16)] - the edge-attr matmul rides along for
    free.  Per chunk, PE does only two matmuls: the merged expand and the
    one-hot segment-reduce; gathered U rows are injected via one
    identity-matmul per 4 chunks.  scatter-mean = PSUM accumulate over the
    window + multiply by 1/max(cnt,1).
"""

import numpy as np
import ml_dtypes

BF16 = ml_dtypes.bfloat16

N = 100_000
E = 3_200_000
D = 128
ED = 16
NCORES = 8
NS = 12_544           # nodes per core
NP = NS * NCORES      # padded node count
WN = 112              # nodes per window (112 + 16 ea rows = 128 = PE K)
NWIN = NS // WN       # 112 windows per core
CHW = 32              # chunks (of 128 edges) per window, uniform
ES = NWIN * CHW * 128  # padded edge slots per core
GRP = 8               # chunks per relu/S-slab/u-inject group
WQ = 4                # windows per srcg/lcol slab load


# ---------------------------------------------------------------------------
# host-side preparation
# ---------------------------------------------------------------------------

def _prep_edges(edge_index, edge_attr):
    """Distribute edges to cores/windows; build per-core slot arrays."""
    src = edge_index[0].astype(np.int64)
    dst = edge_index[1].astype(np.int64)

    cnt = np.bincount(dst, minlength=NP).astype(np.float32)
    invc_full = 1.0 / np.maximum(cnt, 1.0)

    core = dst // NS
    win = (dst % NS) // WN
    l = dst % WN

    per_core = []
    for c in range(NCORES):
        m = core == c
        s_c, w_c, l_c = src[m], win[m], l[m]
        order = np.argsort(w_c, kind="stable")
        s_c, w_c, l_c = s_c[order], w_c[order], l_c[order]
        ea_c = edge_attr[m][order]

        counts = np.bincount(w_c, minlength=NWIN)
        assert counts.max() <= CHW * 128, f"window overflow: {counts.max()}"
        starts = np.concatenate([[0], np.cumsum(counts)])

        # slot arrays, (chunk, partition) order inside each window
        srcg = np.zeros((128, NWIN * CHW), dtype=np.int32)
        lcol = np.full((128, NWIN * CHW), -1.0, dtype=np.float32)
        lrow = np.full((1, ES), -1.0, dtype=np.float32)
        eat = np.zeros((ED, ES), dtype=np.float32)

        for w in range(NWIN):
            k = counts[w]
            sl = slice(starts[w], starts[w + 1])
            j = np.arange(k)
            ch = w * CHW + j // 128
            p = j % 128
            srcg[p, ch] = s_c[sl]
            lcol[p, ch] = l_c[sl]
            pos = ch * 128 + p
            lrow[0, pos] = l_c[sl]
            eat[:, pos] = ea_c[sl].T

        # host-built stacked stationary [S^T(112) ; ea(16)] per edge slot
        stk = np.zeros((128, ES), dtype=BF16)
        lr = lrow[0]
        pos_valid = np.nonzero(lr >= 0)[0]
        stk[lr[pos_valid].astype(np.int64), pos_valid] = 1.0
        stk[WN:] = eat.astype(BF16)

        per_core.append(
            dict(
                srcg=srcg,
                lcol=lcol.astype(BF16),
                lrow=lrow.astype(BF16),
                eat=eat.astype(BF16),
                stk=stk,
                invc=invc_full[c * NS:(c + 1) * NS].reshape(NWIN, WN).T.copy(),
            )
        )
    return per_core


def _prep_inputs(inputs):
    x = inputs["x"]
    xp = np.zeros((NP, D), dtype=np.float32)
    xp[:N] = x
    per_core_edges = _prep_edges(np.asarray(inputs["edge_index"]),
                                 np.asarray(inputs["edge_attr"]))

    iotar = np.broadcast_to(np.arange(WN, dtype=np.float32), (128, WN))
    iotac = np.arange(WN, dtype=np.float32).reshape(WN, 1)
    ones1 = np.ones((1, 128), dtype=np.float32)

    blocks = []
    for i in range(1, 5):
        We = np.asarray(inputs[f"We{i}"], np.float32)
        be = np.asarray(inputs[f"be{i}"], np.float32)
        Wn = np.asarray(inputs[f"Wn{i}"], np.float32)
        bn = np.asarray(inputs[f"bn{i}"], np.float32)
        din = We.shape[0] - ED
        din //= 2
        dout = We.shape[1]
        WeS, WeD, WeE = We[:din], We[din:2 * din], We[2 * din:]
        # pad dout -> 128
        wesd = np.zeros((128, 256), np.float32)
        wesd[:din, :dout] = WeS
        wesd[:din, 128:128 + dout] = WeD
        berow = np.zeros((1, 256), np.float32)
        berow[0, 128:128 + dout] = be
        wee = np.zeros((ED, 128), np.float32)
        wee[:, :dout] = WeE
        wnt = np.zeros((128, dout), np.float32)
        wnt[:din] = Wn[:din]
        wnb = np.zeros((128, dout), np.float32)
        wnb[:dout] = Wn[din:]
        bncol = bn.reshape(dout, 1).astype(np.float32)
        blocks.append(dict(wesd=wesd, berow=berow.astype(BF16),
                           wee=wee.astype(BF16), wnt=wnt,
                           wnb=wnb.astype(BF16), bn=bncol))

    in_maps = []
    for c in range(NCORES):
        m = dict(
            xT=xp[c * NS:(c + 1) * NS].T.copy(),
            srcg=per_core_edges[c]["srcg"],
            lcol=per_core_edges[c]["lcol"],
            lrow=per_core_edges[c]["lrow"],
            eat=per_core_edges[c]["eat"],
            stk=per_core_edges[c]["stk"],
            invc=per_core_edges[c]["invc"],
            iotar=iotar.astype(BF16),
            iotac=iotac.astype(np.float32),
            ones1=ones1.astype(BF16),
        )
        for i, b in enumerate(blocks, 1):
            for k, v in b.items():
                m[f"{k}{i}"] = v
        in_maps.append(m)
    return in_maps


# ---------------------------------------------------------------------------
# numpy emulation of the device dataflow (for fast correctness checking)
# ---------------------------------------------------------------------------

def _emulate(in_maps):
    import jax

    f32 = np.float32
    outs = []
    HT = [m["xT"].astype(f32).copy() for m in in_maps]
    for i in range(1, 5):
        Us, Vs = [], []
        for c, m in enumerate(in_maps):
            wesd = m[f"wesd{i}"].astype(f32)
            uv = HT[c].T @ wesd  # [NS, 256]
            uv += np.ones((NS, 1), f32) @ m[f"berow{i}"].astype(f32)
            Us.append(uv[:, :128].astype(BF16))
            Vs.append(uv[:, 128:].astype(BF16))
        U_full = np.concatenate(Us, 0)  # bf16 allgather
        for c, m in enumerate(in_maps):
            V = Vs[c].astype(f32)
            # slot s = g*128 + p  (g = global chunk)
            src_s = m["srcg"].T.reshape(-1)
            l_s = m["lcol"].T.reshape(-1).astype(f32)
            valid = l_s >= 0
            win_s = np.arange(ES) // (CHW * 128)
            dst_s = (win_s * WN + l_s.astype(np.int64).clip(0))
            u = U_full[src_s].astype(f32)
            v = np.where(valid[:, None], V[dst_s], 0.0)
            ew = m["eat"].T.astype(f32) @ m[f"wee{i}"].astype(f32)
            msg = np.maximum(u + v + ew, 0).astype(BF16).astype(f32)
            agg = np.array(jax.ops.segment_sum(
                msg[valid], dst_s[valid], num_segments=NS))
            agg *= m["invc"].T.reshape(-1)[:, None]
            aggT = agg.astype(BF16).astype(f32).T
            hT = m[f"wnt{i}"].astype(f32).T @ HT[c]
            hT += m[f"wnb{i}"].astype(f32).T @ aggT
            hT += m[f"bn{i}"].astype(f32)
            if i < 4:
                HT[c] = np.maximum(hT, 0)
            else:
                outs.append(1.0 / (1.0 + np.exp(-hT[0])))
    return np.concatenate(outs)[:N].reshape(N, 1).astype(np.float32)


# ---------------------------------------------------------------------------
# bass program
# ---------------------------------------------------------------------------

def _build():
    from concourse import bacc, bass, mybir, tile
    from concourse.masks import make_identity

    f32 = mybir.dt.float32
    bf16 = mybir.dt.bfloat16
    i32 = mybir.dt.int32

    nc = bacc.Bacc("TRN2", num_devices=NCORES)

    inp = {}
    for name, shape, dt in [
        ("xT", [128, NS], f32),
        ("srcg", [128, NWIN * CHW], i32),
        ("lcol", [128, NWIN * CHW], bf16),
        ("stk", [128, ES], bf16),
        ("invc", [WN, NWIN], f32),
        ("iotar", [128, WN], bf16),
        ("ones1", [1, 128], bf16),
    ]:
        inp[name] = nc.dram_tensor(name, shape, dt, kind="ExternalInput")
    for i in range(1, 5):
        dout = 1 if i == 4 else D
        for name, shape, dt in [
            (f"wesd{i}", [128, 256], f32),
            (f"berow{i}", [1, 256], bf16),
            (f"wee{i}", [ED, 128], bf16),
            (f"wnt{i}", [128, dout], f32),
            (f"wnb{i}", [128, dout], bf16),
            (f"bn{i}", [dout, 1], f32),
        ]:
            inp[name] = nc.dram_tensor(name, shape, dt, kind="ExternalInput")
    out_ext = nc.dram_tensor("out", [1, NS], f32, kind="ExternalOutput")

    with tile.TileContext(nc) as tc:
        with tc.tile_pool(name="res", bufs=1) as res, \
             tc.tile_pool(name="sb", bufs=2) as sb, \
             tc.tile_pool(name="pbig", bufs=2, space="PSUM") as pbig, \
             tc.tile_pool(name="psmall", bufs=2, space="PSUM") as psmall, \
             tc.tile_pool(name="dram", bufs=1, space="DRAM") as dram:

            # resident tensors
            hT = res.tile([128, NS], f32)          # node features, transposed
            # [V_win(112) ; WeE(16)] per window, window w at cols w*128
            vw = res.tile([128, NWIN * 128], bf16)
            aggT = res.tile([128, NS], bf16)
            invc_sb = res.tile([WN, NWIN], f32)
            iotar_sb = res.tile([128, WN], bf16)
            ones_sb = res.tile([1, 128], bf16)
            ident = res.tile([128, 128], bf16)

            nc.sync.dma_start(hT[:], inp["xT"][:])
            nc.sync.dma_start(invc_sb[:], inp["invc"][:])
            nc.sync.dma_start(iotar_sb[:], inp["iotar"][:])
            nc.sync.dma_start(ones_sb[:], inp["ones1"][:])
            make_identity(nc, ident[:])

            u_bounce = dram.tile([NS, 128], bf16)

            for i in range(1, 5):
                dout = 1 if i == 4 else D
                u_full = dram.tile([NP, 128], bf16, addr_space="Shared",
                                   name=f"u_full{i}", tag=f"u_full{i}")
                wesd = sb.tile([128, 256], f32, tag="wesd")
                berow = sb.tile([1, 256], bf16, tag="berow")
                wnt = sb.tile([128, dout], f32, tag="wnt")
                wnb = sb.tile([128, dout], bf16, tag="wnb")
                bncol = sb.tile([dout, 1], f32, tag="bncol")
                nc.sync.dma_start(wesd[:], inp[f"wesd{i}"][:])
                nc.sync.dma_start(berow[:], inp[f"berow{i}"][:])
                nc.sync.dma_start(wnt[:], inp[f"wnt{i}"][:])
                nc.sync.dma_start(wnb[:], inp[f"wnb{i}"][:])
                nc.sync.dma_start(bncol[:], inp[f"bn{i}"][:])
                # WeE replicated into rows 112:128 of every window block
                nc.sync.dma_start(
                    vw[WN:128, :].rearrange("p (w d) -> p w d", w=NWIN),
                    inp[f"wee{i}"][:, None, :].to_broadcast([ED, NWIN, 128]))

                # ---- node-side: U,V tiles (row-major) ----
                for t in range(NWIN):
                    puv = pbig.tile([128, 512], f32, tag="pbig")
                    nc.tensor.matmul(out=puv[:WN, :256],
                                     lhsT=hT[:, t * WN:(t + 1) * WN],
                                     rhs=wesd[:], start=True, stop=False)
                    nc.tensor.matmul(out=puv[:WN, :256],
                                     lhsT=ones_sb[:, :WN],
                                     rhs=berow[:], start=False, stop=True)
                    utile = sb.tile([WN, 128], bf16, tag="utile")
                    nc.scalar.copy(utile[:], puv[:WN, :128])
                    nc.vector.tensor_copy(vw[:WN, t * 128:(t + 1) * 128],
                                          puv[:WN, 128:256])
                    nc.sync.dma_start(u_bounce[t * WN:(t + 1) * WN, :],
                                      utile[:])

                nc.gpsimd.collective_compute(
                    "AllGather", mybir.AluOpType.bypass,
                    replica_groups=[list(range(NCORES))],
                    ins=[u_bounce.opt()], outs=[u_full.opt()],
                )

                # ---- edge phase ----
                for w in range(NWIN):
                    if w % WQ == 0:
                        srcg_sl = sb.tile([128, WQ * CHW], i32, tag="srcg")
                        nc.sync.dma_start(
                            srcg_sl[:],
                            inp["srcg"][:, w * CHW:(w + WQ) * CHW])
                        lcol_sl = sb.tile([128, WQ * CHW], bf16, tag="lcolw")
                        nc.sync.dma_start(
                            lcol_sl[:],
                            inp["lcol"][:, w * CHW:(w + WQ) * CHW])
                    w0 = (w % WQ) * CHW
                    uslab = sb.tile([128, CHW * 128], bf16, tag="uslab")
                    nc.gpsimd.indirect_dma_start(
                        out=uslab[:],
                        out_offset=None,
                        in_=u_full[:],
                        in_offset=bass.IndirectOffsetOnAxis(
                            ap=srcg_sl[:, w0:w0 + CHW], axis=0),
                    )
                    # stacked stationary [S^T(112) ; ea(16)] for the window
                    stslab = sb.tile([128, CHW * 128], bf16, tag="stslab")
                    nc.sync.dma_start(
                        stslab[:],
                        inp["stk"][:, w * CHW * 128:(w + 1) * CHW * 128])

                    pw = psmall.tile([128, 128], f32, tag="pw")
                    for g0 in range(0, CHW, GRP):
                        sslab = sb.tile([128, GRP * WN], bf16, tag="sslab")
                        lc3 = lcol_sl[:, w0 + g0:w0 + g0 + GRP]
                        nc.vector.tensor_tensor(
                            out=sslab[:].rearrange("p (c e) -> p c e", c=GRP),
                            in0=lc3[:, :, None].to_broadcast([128, GRP, WN]),
                            in1=iotar_sb[:, None, :].to_broadcast(
                                [128, GRP, WN]),
                            op=mybir.AluOpType.is_equal)

                        pe_ = pbig.tile([128, GRP * 128], f32, tag="pbig")
                        for h in range(0, GRP, 4):
                            nc.tensor.matmul(
                                out=pe_[:, h * 128:(h + 4) * 128],
                                lhsT=ident[:],
                                rhs=uslab[:, (g0 + h) * 128:
                                          (g0 + h + 4) * 128],
                                start=True, stop=False)
                        for c in range(g0, g0 + GRP):
                            r = (c - g0) * 128
                            nc.tensor.matmul(
                                out=pe_[:, r:r + 128],
                                lhsT=stslab[:, c * 128:(c + 1) * 128],
                                rhs=vw[:, w * 128:(w + 1) * 128],
                                start=False, stop=True)
                        wslab = sb.tile([128, GRP * 128], bf16, tag="wslab")
                        nc.scalar.activation(
                            wslab[:], pe_[:],
                            mybir.ActivationFunctionType.Relu)
                        for c in range(g0, g0 + GRP):
                            r = (c - g0) * 128
                            nc.tensor.matmul(
                                out=pw[:WN, :],
                                lhsT=sslab[:, r // 128 * WN:
                                           (r // 128 + 1) * WN],
                                rhs=wslab[:, r:r + 128],
                                start=(c == 0), stop=(c == CHW - 1))
                    # scatter-mean + transpose into aggT
                    argm = sb.tile([WN, 128], bf16, tag="argm")
                    nc.vector.tensor_scalar(
                        out=argm[:], in0=pw[:WN, :],
                        scalar1=invc_sb[:, w:w + 1], scalar2=None,
                        op0=mybir.AluOpType.mult)
                    pt = psmall.tile([128, WN], bf16, tag="pt")
                    nc.tensor.transpose(pt[:], argm[:], ident[:WN, :WN])
                    nc.scalar.copy(aggT[:, w * WN:(w + 1) * WN], pt[:])

                # ---- node update ----
                nsz = [512] * (NS // 512) + ([NS % 512] if NS % 512 else [])
                off = 0
                for sz in nsz:
                    ph = pbig.tile([128, 512], f32, tag="pbig")
                    nc.tensor.matmul(out=ph[:dout, :sz], lhsT=wnt[:],
                                     rhs=hT[:, off:off + sz],
                                     start=True, stop=False)
                    nc.tensor.matmul(out=ph[:dout, :sz], lhsT=wnb[:],
                                     rhs=aggT[:, off:off + sz],
                                     start=False, stop=True)
                    if i < 4:
                        nc.scalar.activation(
                            hT[:, off:off + sz], ph[:, :sz],
                            mybir.ActivationFunctionType.Relu,
                            bias=bncol[:])
                    else:
                        out_t = sb.tile([1, 512], f32, tag="out_t")
                        nc.scalar.activation(
                            out_t[:, :sz], ph[:dout, :sz],
                            mybir.ActivationFunctionType.Sigmoid,
                            bias=bncol[:])
                        nc.sync.dma_start(out_ext[:, off:off + sz],
                                          out_t[:, :sz])
                    off += sz

    nc.finalize()
    return nc


_NC_CACHE = {}


def kernel(**inputs):
    from concourse.bass_utils import run_bass_kernel_spmd

    in_maps = _prep_inputs(inputs)
    if "nc" not in _NC_CACHE:
        _NC_CACHE["nc"] = _build()
    nc = _NC_CACHE["nc"]
    res = run_bass_kernel_spmd(nc, in_maps, core_ids=list(range(NCORES)))
    outs = [res.results[c]["out"].reshape(-1) for c in range(NCORES)]
    return np.concatenate(outs)[:N].reshape(N, 1).astype(np.float32)


# revision 35
# speedup vs baseline: 2.9008x; 1.0082x over previous
"""Distributed Trainium2 kernel for the 4-block GNN (nn_ActorGNN).

Strategy (edge-parallel, dst-sharded):
  - Pad N=100000 -> NP=100352 = 8 * 12544 nodes; core c owns nodes
    [c*12544, (c+1)*12544).  Node features live transposed in SBUF (H^T).
  - Algebra: the edge MLP  relu([x_src|x_dst|ea] @ We + be)  is split as
    relu(U[src] + V[dst] + ea@WeE + be) with U = x@WeS, V = x@WeD computed
    per node shard (cheap N-side matmuls).
  - U is AllGathered (bf16); every core gathers arbitrary source rows with
    SWDGE indirect DMA; V/agg stay core-local (edges live on the core that
    owns their destination).
  - Edges are grouped by destination window of 112 nodes and padded to a
    uniform 32 chunks x 128 edges per window (SPMD-uniform).  112 was
    chosen so the expand matmul's contraction packs [S^T(112) ; ea(16)]
    against [V_win(112) ; WeE(16)] - the edge-attr matmul rides along for
    free.  Per chunk, PE does only two matmuls: the merged expand and the
    one-hot segment-reduce; gathered U rows are injected via one
    identity-matmul per 4 chunks.  scatter-mean = PSUM accumulate over the
    window + multiply by 1/max(cnt,1).
"""

import numpy as np
import ml_dtypes

BF16 = ml_dtypes.bfloat16

N = 100_000
E = 3_200_000
D = 128
ED = 16
NCORES = 8
NS = 12_544           # nodes per core
NP = NS * NCORES      # padded node count
WN = 112              # nodes per window (112 + 16 ea rows = 128 = PE K)
NWIN = NS // WN       # 112 windows per core
CHW = 32              # chunks (of 128 edges) per window, uniform
ES = NWIN * CHW * 128  # padded edge slots per core
GRP = 8               # chunks per relu/S-slab/u-inject group
WQ = 4                # windows per srcg/lcol slab load


# ---------------------------------------------------------------------------
# host-side preparation
# ---------------------------------------------------------------------------

def _prep_edges(edge_index, edge_attr):
    """Distribute edges to cores/windows; build per-core slot arrays."""
    src = edge_index[0].astype(np.int64)
    dst = edge_index[1].astype(np.int64)

    cnt = np.bincount(dst, minlength=NP).astype(np.float32)
    invc_full = 1.0 / np.maximum(cnt, 1.0)

    core = dst // NS
    win = (dst % NS) // WN
    l = dst % WN

    per_core = []
    for c in range(NCORES):
        m = core == c
        s_c, w_c, l_c = src[m], win[m], l[m]
        order = np.argsort(w_c, kind="stable")
        s_c, w_c, l_c = s_c[order], w_c[order], l_c[order]
        ea_c = edge_attr[m][order]

        counts = np.bincount(w_c, minlength=NWIN)
        assert counts.max() <= CHW * 128, f"window overflow: {counts.max()}"
        starts = np.concatenate([[0], np.cumsum(counts)])

        # slot arrays, (chunk, partition) order inside each window
        srcg = np.zeros((128, NWIN * CHW), dtype=np.int32)
        lcol = np.full((128, NWIN * CHW), -1.0, dtype=np.float32)
        lrow = np.full((1, ES), -1.0, dtype=np.float32)
        eat = np.zeros((ED, ES), dtype=np.float32)

        for w in range(NWIN):
            k = counts[w]
            sl = slice(starts[w], starts[w + 1])
            j = np.arange(k)
            ch = w * CHW + j // 128
            p = j % 128
            srcg[p, ch] = s_c[sl]
            lcol[p, ch] = l_c[sl]
            pos = ch * 128 + p
            lrow[0, pos] = l_c[sl]
            eat[:, pos] = ea_c[sl].T

        # host-built stacked stationary [S^T(112) ; ea(16)] per edge slot
        stk = np.zeros((128, ES), dtype=BF16)
        lr = lrow[0]
        pos_valid = np.nonzero(lr >= 0)[0]
        stk[lr[pos_valid].astype(np.int64), pos_valid] = 1.0
        stk[WN:] = eat.astype(BF16)

        per_core.append(
            dict(
                srcg=srcg,
                lcol=lcol.astype(BF16),
                lrow=lrow.astype(BF16),
                eat=eat.astype(BF16),
                stk=stk,
                invc=invc_full[c * NS:(c + 1) * NS].reshape(NWIN, WN).T.copy(),
            )
        )
    return per_core


def _prep_inputs(inputs):
    x = inputs["x"]
    xp = np.zeros((NP, D), dtype=np.float32)
    xp[:N] = x
    per_core_edges = _prep_edges(np.asarray(inputs["edge_index"]),
                                 np.asarray(inputs["edge_attr"]))

    iotar = np.broadcast_to(np.arange(WN, dtype=np.float32), (128, WN))
    iotac = np.arange(WN, dtype=np.float32).reshape(WN, 1)
    ones1 = np.ones((1, 128), dtype=np.float32)

    blocks = []
    for i in range(1, 5):
        We = np.asarray(inputs[f"We{i}"], np.float32)
        be = np.asarray(inputs[f"be{i}"], np.float32)
        Wn = np.asarray(inputs[f"Wn{i}"], np.float32)
        bn = np.asarray(inputs[f"bn{i}"], np.float32)
        din = We.shape[0] - ED
        din //= 2
        dout = We.shape[1]
        WeS, WeD, WeE = We[:din], We[din:2 * din], We[2 * din:]
        # pad dout -> 128
        wesd = np.zeros((128, 256), np.float32)
        wesd[:din, :dout] = WeS
        wesd[:din, 128:128 + dout] = WeD
        berow = np.zeros((1, 256), np.float32)
        berow[0, 128:128 + dout] = be
        wee = np.zeros((ED, 128), np.float32)
        wee[:, :dout] = WeE
        wnt = np.zeros((128, dout), np.float32)
        wnt[:din] = Wn[:din]
        wnb = np.zeros((128, dout), np.float32)
        wnb[:dout] = Wn[din:]
        bncol = bn.reshape(dout, 1).astype(np.float32)
        b = dict(wesd=wesd, berow=berow.astype(BF16),
                 wee=wee.astype(BF16), wnt=wnt,
                 wnb=wnb.astype(BF16), bn=bncol)
        if i == 4:
            # slim block-4 params: dout=1, keep only the real column
            wesd4b = np.zeros((128, 2), np.float32)
            wesd4b[:din, 0] = WeS[:, 0]
            wesd4b[:din, 1] = WeD[:, 0]
            berow4b = np.zeros((1, 2), np.float32)
            berow4b[0, 1] = be[0]
            b["wesdb"] = wesd4b
            b["berowb"] = berow4b.astype(BF16)
            b["weeb"] = np.tile(WeE[:, :1], (1, NWIN)).astype(BF16)
        blocks.append(b)

    in_maps = []
    for c in range(NCORES):
        m = dict(
            xT=xp[c * NS:(c + 1) * NS].T.copy(),
            srcg=per_core_edges[c]["srcg"],
            lcol=per_core_edges[c]["lcol"],
            lrow=per_core_edges[c]["lrow"],
            eat=per_core_edges[c]["eat"],
            stk=per_core_edges[c]["stk"],
            invc=per_core_edges[c]["invc"],
            iotar=iotar.astype(BF16),
            iotac=iotac.astype(np.float32),
            ones1=ones1.astype(BF16),
        )
        for i, b in enumerate(blocks, 1):
            for k, v in b.items():
                m[f"{k}{i}"] = v
        in_maps.append(m)
    return in_maps


# ---------------------------------------------------------------------------
# numpy emulation of the device dataflow (for fast correctness checking)
# ---------------------------------------------------------------------------

def _emulate(in_maps):
    import jax

    f32 = np.float32
    outs = []
    HT = [m["xT"].astype(f32).copy() for m in in_maps]
    for i in range(1, 5):
        Us, Vs = [], []
        for c, m in enumerate(in_maps):
            wesd = m[f"wesd{i}"].astype(f32)
            uv = HT[c].T @ wesd  # [NS, 256]
            uv += np.ones((NS, 1), f32) @ m[f"berow{i}"].astype(f32)
            Us.append(uv[:, :128].astype(BF16))
            Vs.append(uv[:, 128:].astype(BF16))
        U_full = np.concatenate(Us, 0)  # bf16 allgather
        for c, m in enumerate(in_maps):
            V = Vs[c].astype(f32)
            # slot s = g*128 + p  (g = global chunk)
            src_s = m["srcg"].T.reshape(-1)
            l_s = m["lcol"].T.reshape(-1).astype(f32)
            valid = l_s >= 0
            win_s = np.arange(ES) // (CHW * 128)
            dst_s = (win_s * WN + l_s.astype(np.int64).clip(0))
            u = U_full[src_s].astype(f32)
            v = np.where(valid[:, None], V[dst_s], 0.0)
            ew = m["eat"].T.astype(f32) @ m[f"wee{i}"].astype(f32)
            msg = np.maximum(u + v + ew, 0).astype(BF16).astype(f32)
            agg = np.array(jax.ops.segment_sum(
                msg[valid], dst_s[valid], num_segments=NS))
            agg *= m["invc"].T.reshape(-1)[:, None]
            aggT = agg.astype(BF16).astype(f32).T
            hT = m[f"wnt{i}"].astype(f32).T @ HT[c]
            hT += m[f"wnb{i}"].astype(f32).T @ aggT
            hT += m[f"bn{i}"].astype(f32)
            if i < 4:
                HT[c] = np.maximum(hT, 0)
            else:
                outs.append(1.0 / (1.0 + np.exp(-hT[0])))
    return np.concatenate(outs)[:N].reshape(N, 1).astype(np.float32)


# ---------------------------------------------------------------------------
# bass program
# ---------------------------------------------------------------------------

def _build():
    from concourse import bacc, bass, mybir, tile
    from concourse.masks import make_identity

    f32 = mybir.dt.float32
    bf16 = mybir.dt.bfloat16
    i32 = mybir.dt.int32

    nc = bacc.Bacc("TRN2", num_devices=NCORES)

    inp = {}
    for name, shape, dt in [
        ("xT", [128, NS], f32),
        ("srcg", [128, NWIN * CHW], i32),
        ("lcol", [128, NWIN * CHW], bf16),
        ("stk", [128, ES], bf16),
        ("invc", [WN, NWIN], f32),
        ("iotar", [128, WN], bf16),
        ("ones1", [1, 128], bf16),
    ]:
        inp[name] = nc.dram_tensor(name, shape, dt, kind="ExternalInput")
    for i in range(1, 5):
        dout = 1 if i == 4 else D
        for name, shape, dt in [
            (f"wesd{i}", [128, 256], f32),
            (f"berow{i}", [1, 256], bf16),
            (f"wee{i}", [ED, 128], bf16),
            (f"wnt{i}", [128, dout], f32),
            (f"wnb{i}", [128, dout], bf16),
            (f"bn{i}", [dout, 1], f32),
        ]:
            inp[name] = nc.dram_tensor(name, shape, dt, kind="ExternalInput")
    for name, shape, dt in [
        ("wesdb4", [128, 2], f32),
        ("berowb4", [1, 2], bf16),
        ("weeb4", [ED, NWIN], bf16),
    ]:
        inp[name] = nc.dram_tensor(name, shape, dt, kind="ExternalInput")
    out_ext = nc.dram_tensor("out", [1, NS], f32, kind="ExternalOutput")

    with tile.TileContext(nc) as tc:
        with tc.tile_pool(name="res", bufs=1) as res, \
             tc.tile_pool(name="sb", bufs=2) as sb, \
             tc.tile_pool(name="pbig", bufs=2, space="PSUM") as pbig, \
             tc.tile_pool(name="psmall", bufs=2, space="PSUM") as psmall, \
             tc.tile_pool(name="dram", bufs=1, space="DRAM") as dram:

            # resident tensors
            hT = res.tile([128, NS], f32)          # node features, transposed
            # [V_win(112) ; WeE(16)] per window, window w at cols w*128
            vw = res.tile([128, NWIN * 128], bf16)
            aggT = res.tile([128, NS], bf16)
            invc_sb = res.tile([WN, NWIN], f32)
            iotar_sb = res.tile([128, WN], bf16)
            ones_sb = res.tile([1, 128], bf16)
            ident = res.tile([128, 128], bf16)

            nc.sync.dma_start(hT[:], inp["xT"][:])
            nc.sync.dma_start(invc_sb[:], inp["invc"][:])
            nc.sync.dma_start(iotar_sb[:], inp["iotar"][:])
            nc.sync.dma_start(ones_sb[:], inp["ones1"][:])
            make_identity(nc, ident[:])

            u_bounce = dram.tile([NS, 128], bf16)

            u4_bounce = dram.tile([NS, 2], bf16, name="u4_bounce",
                                  tag="u4_bounce")
            for i in range(1, 5):
                dout = 1 if i == 4 else D
                slim = i == 4
                wnt = sb.tile([128, dout], f32, tag="wnt")
                wnb = sb.tile([128, dout], bf16, tag="wnb")
                bncol = sb.tile([dout, 1], f32, tag="bncol")
                nc.sync.dma_start(wnt[:], inp[f"wnt{i}"][:])
                nc.sync.dma_start(wnb[:], inp[f"wnb{i}"][:])
                nc.sync.dma_start(bncol[:], inp[f"bn{i}"][:])
                if not slim:
                    u_full = dram.tile([NP, 128], bf16, addr_space="Shared",
                                       name=f"u_full{i}", tag=f"u_full{i}")
                    wesd = sb.tile([128, 256], f32, tag="wesd")
                    berow = sb.tile([1, 256], bf16, tag="berow")
                    nc.sync.dma_start(wesd[:], inp[f"wesd{i}"][:])
                    nc.sync.dma_start(berow[:], inp[f"berow{i}"][:])
                    # WeE replicated into rows 112:128 of every window block
                    nc.sync.dma_start(
                        vw[WN:128, :].rearrange("p (w d) -> p w d", w=NWIN),
                        inp[f"wee{i}"][:, None, :]
                        .to_broadcast([ED, NWIN, 128]))
                else:
                    u_full = dram.tile([NP, 2], bf16, addr_space="Shared",
                                       name="u4_full", tag="u4_full")
                    wesd = sb.tile([128, 2], f32, tag="wesdb")
                    berow = sb.tile([1, 2], bf16, tag="berowb")
                    nc.sync.dma_start(wesd[:], inp["wesdb4"][:])
                    nc.sync.dma_start(berow[:], inp["berowb4"][:])
                    vw4 = res.tile([128, NWIN], bf16)
                    nc.sync.dma_start(vw4[WN:128, :],
                                      inp["weeb4"][:])

                # ---- node-side: U,V tiles (row-major) ----
                nuv = 2 if slim else 256
                for t in range(NWIN):
                    puv = pbig.tile([128, 1024], f32, tag="pbig")
                    nc.tensor.matmul(out=puv[:WN, :nuv],
                                     lhsT=hT[:, t * WN:(t + 1) * WN],
                                     rhs=wesd[:], start=True, stop=False)
                    nc.tensor.matmul(out=puv[:WN, :nuv],
                                     lhsT=ones_sb[:, :WN],
                                     rhs=berow[:], start=False, stop=True)
                    if not slim:
                        utile = sb.tile([WN, 128], bf16, tag="utile")
                        nc.scalar.copy(utile[:], puv[:WN, :128])
                        nc.vector.tensor_copy(vw[:WN, t * 128:(t + 1) * 128],
                                              puv[:WN, 128:256])
                        nc.sync.dma_start(u_bounce[t * WN:(t + 1) * WN, :],
                                          utile[:])
                    else:
                        utile = sb.tile([WN, 2], bf16, tag="utile4")
                        nc.scalar.copy(utile[:], puv[:WN, :2])
                        nc.vector.tensor_copy(vw4[:WN, t:t + 1],
                                              puv[:WN, 1:2])
                        nc.sync.dma_start(u4_bounce[t * WN:(t + 1) * WN, :],
                                          utile[:])

                nc.gpsimd.collective_compute(
                    "AllGather", mybir.AluOpType.bypass,
                    replica_groups=[list(range(NCORES))],
                    ins=[(u4_bounce if slim else u_bounce).opt()],
                    outs=[u_full.opt()],
                )

                # ---- edge phase ----
                for w in range(NWIN):
                    if w % WQ == 0:
                        srcg_sl = sb.tile([128, WQ * CHW], i32, tag="srcg")
                        nc.sync.dma_start(
                            srcg_sl[:],
                            inp["srcg"][:, w * CHW:(w + WQ) * CHW])
                        lcol_sl = sb.tile([128, WQ * CHW], bf16, tag="lcolw")
                        nc.sync.dma_start(
                            lcol_sl[:],
                            inp["lcol"][:, w * CHW:(w + WQ) * CHW])
                    w0 = (w % WQ) * CHW
                    uslab = sb.tile(
                        [128, CHW * 2] if slim else [128, CHW * 128],
                        bf16, tag="uslab4" if slim else "uslab")
                    nc.gpsimd.indirect_dma_start(
                        out=uslab[:],
                        out_offset=None,
                        in_=u_full[:],
                        in_offset=bass.IndirectOffsetOnAxis(
                            ap=srcg_sl[:, w0:w0 + CHW], axis=0),
                    )
                    # stacked stationary [S^T(112) ; ea(16)] for the window
                    stslab = sb.tile([128, CHW * 128], bf16, tag="stslab")
                    nc.sync.dma_start(
                        stslab[:],
                        inp["stk"][:, w * CHW * 128:(w + 1) * CHW * 128])

                    pw = psmall.tile([128, 128], f32, tag="pw")
                    for g0 in range(0, CHW, GRP):
                        sslab = sb.tile([128, GRP * WN], bf16, tag="sslab")
                        lc3 = lcol_sl[:, w0 + g0:w0 + g0 + GRP]
                        nc.vector.tensor_tensor(
                            out=sslab[:].rearrange("p (c e) -> p c e", c=GRP),
                            in0=lc3[:, :, None].to_broadcast([128, GRP, WN]),
                            in1=iotar_sb[:, None, :].to_broadcast(
                                [128, GRP, WN]),
                            op=mybir.AluOpType.is_equal)

                        if not slim:
                            pe_ = pbig.tile([128, GRP * 128], f32, tag="pbig")
                            for h in range(0, GRP, 4):
                                nc.tensor.matmul(
                                    out=pe_[:, h * 128:(h + 4) * 128],
                                    lhsT=ident[:],
                                    rhs=uslab[:, (g0 + h) * 128:
                                              (g0 + h + 4) * 128],
                                    start=True, stop=False)
                            for c in range(g0, g0 + GRP):
                                r = (c - g0) * 128
                                nc.tensor.matmul(
                                    out=pe_[:, r:r + 128],
                                    lhsT=stslab[:, c * 128:(c + 1) * 128],
                                    rhs=vw[:, w * 128:(w + 1) * 128],
                                    start=False, stop=True)
                            wslab = sb.tile([128, GRP * 128], bf16,
                                            tag="wslab")
                            nc.scalar.activation(
                                wslab[:], pe_[:],
                                mybir.ActivationFunctionType.Relu)
                            for c in range(g0, g0 + GRP):
                                r = (c - g0) * 128
                                nc.tensor.matmul(
                                    out=pw[:WN, :],
                                    lhsT=sslab[:, r // 128 * WN:
                                               (r // 128 + 1) * WN],
                                    rhs=wslab[:, r:r + 128],
                                    start=(c == 0), stop=(c == CHW - 1))
                        else:
                            pe4 = pbig.tile([128, GRP], f32, tag="pbig")
                            for c in range(g0, g0 + GRP):
                                nc.tensor.matmul(
                                    out=pe4[:, c - g0:c - g0 + 1],
                                    lhsT=stslab[:, c * 128:(c + 1) * 128],
                                    rhs=vw4[:, w:w + 1],
                                    start=True, stop=True)
                            u4r = uslab[:].rearrange(
                                "p (c t) -> p c t", t=2)
                            nc.vector.tensor_tensor(
                                out=pe4[:], in0=pe4[:],
                                in1=u4r[:, g0:g0 + GRP, 0:1],
                                op=mybir.AluOpType.add)
                            wslab4 = sb.tile([128, GRP], bf16, tag="wslab4")
                            nc.scalar.activation(
                                wslab4[:], pe4[:],
                                mybir.ActivationFunctionType.Relu)
                            for c in range(g0, g0 + GRP):
                                cc = c - g0
                                nc.tensor.matmul(
                                    out=pw[:WN, :1],
                                    lhsT=sslab[:, cc * WN:(cc + 1) * WN],
                                    rhs=wslab4[:, cc:cc + 1],
                                    start=(c == 0), stop=(c == CHW - 1))
                    # scatter-mean + transpose into aggT
                    nd = 1 if slim else 128
                    argm = sb.tile([WN, nd], bf16,
                                   tag="argm4" if slim else "argm")
                    nc.vector.tensor_scalar(
                        out=argm[:], in0=pw[:WN, :nd],
                        scalar1=invc_sb[:, w:w + 1], scalar2=None,
                        op0=mybir.AluOpType.mult)
                    pt = psmall.tile([nd, WN], bf16, tag="pt")
                    nc.tensor.transpose(pt[:], argm[:], ident[:WN, :WN])
                    nc.scalar.copy(aggT[:nd, w * WN:(w + 1) * WN], pt[:])

                # ---- node update ----
                nsz = [512] * (NS // 512) + ([NS % 512] if NS % 512 else [])
                off = 0
                for sz in nsz:
                    ph = pbig.tile([128, 512], f32, tag="pbig")
                    nc.tensor.matmul(out=ph[:dout, :sz], lhsT=wnt[:],
                                     rhs=hT[:, off:off + sz],
                                     start=True, stop=False)
                    nc.tensor.matmul(out=ph[:dout, :sz], lhsT=wnb[:],
                                     rhs=aggT[:, off:off + sz],
                                     start=False, stop=True)
                    if i < 4:
                        nc.scalar.activation(
                            hT[:, off:off + sz], ph[:, :sz],
                            mybir.ActivationFunctionType.Relu,
                            bias=bncol[:])
                    else:
                        out_t = sb.tile([1, 512], f32, tag="out_t")
                        nc.scalar.activation(
                            out_t[:, :sz], ph[:dout, :sz],
                            mybir.ActivationFunctionType.Sigmoid,
                            bias=bncol[:])
                        nc.sync.dma_start(out_ext[:, off:off + sz],
                                          out_t[:, :sz])
                    off += sz

    nc.finalize()
    return nc


_NC_CACHE = {}


def kernel(**inputs):
    from concourse.bass_utils import run_bass_kernel_spmd

    in_maps = _prep_inputs(inputs)
    if "nc" not in _NC_CACHE:
        _NC_CACHE["nc"] = _build()
    nc = _NC_CACHE["nc"]
    res = run_bass_kernel_spmd(nc, in_maps, core_ids=list(range(NCORES)))
    outs = [res.results[c]["out"].reshape(-1) for c in range(NCORES)]
    return np.concatenate(outs)[:N].reshape(N, 1).astype(np.float32)


# revision 36
# speedup vs baseline: 3.1669x; 1.0917x over previous
"""Distributed Trainium2 kernel for the 4-block GNN (nn_ActorGNN).

Strategy (edge-parallel, dst-sharded):
  - Pad N=100000 -> NP=100352 = 8 * 12544 nodes; core c owns nodes
    [c*12544, (c+1)*12544).  Node features live transposed in SBUF (H^T).
  - Algebra: the edge MLP  relu([x_src|x_dst|ea] @ We + be)  is split as
    relu(U[src] + V[dst] + ea@WeE + be) with U = x@WeS, V = x@WeD computed
    per node shard (cheap N-side matmuls).
  - U is AllGathered (bf16); every core gathers arbitrary source rows with
    SWDGE indirect DMA; V/agg stay core-local (edges live on the core that
    owns their destination).
  - Edges are grouped by destination window of 112 nodes and padded to a
    uniform 32 chunks x 128 edges per window (SPMD-uniform).  112 was
    chosen so the expand matmul's contraction packs [S^T(112) ; ea(16)]
    against [V_win(112) ; WeE(16)] - the edge-attr matmul rides along for
    free.  Per chunk, PE does only two matmuls: the merged expand and the
    one-hot segment-reduce; gathered U rows are injected via one
    identity-matmul per 4 chunks.  scatter-mean = PSUM accumulate over the
    window + multiply by 1/max(cnt,1).
"""

import numpy as np
import ml_dtypes

BF16 = ml_dtypes.bfloat16

N = 100_000
E = 3_200_000
D = 128
ED = 16
NCORES = 8
NS = 12_544           # nodes per core
NP = NS * NCORES      # padded node count
WN = 112              # nodes per window (112 + 16 ea rows = 128 = PE K)
NWIN = NS // WN       # 112 windows per core
CHW = 32              # chunks (of 128 edges) per window, uniform
ES = NWIN * CHW * 128  # padded edge slots per core
GRP = 8               # chunks per relu/S-slab/u-inject group
WQ = 4                # windows per srcg/lcol slab load


# ---------------------------------------------------------------------------
# host-side preparation
# ---------------------------------------------------------------------------

def _prep_edges(edge_index, edge_attr):
    """Distribute edges to cores/windows; build per-core slot arrays."""
    src = edge_index[0].astype(np.int64)
    dst = edge_index[1].astype(np.int64)

    cnt = np.bincount(dst, minlength=NP).astype(np.float32)
    invc_full = 1.0 / np.maximum(cnt, 1.0)

    core = dst // NS
    win = (dst % NS) // WN
    l = dst % WN

    per_core = []
    for c in range(NCORES):
        m = core == c
        s_c, w_c, l_c = src[m], win[m], l[m]
        order = np.argsort(w_c, kind="stable")
        s_c, w_c, l_c = s_c[order], w_c[order], l_c[order]
        ea_c = edge_attr[m][order]

        counts = np.bincount(w_c, minlength=NWIN)
        assert counts.max() <= CHW * 128, f"window overflow: {counts.max()}"
        starts = np.concatenate([[0], np.cumsum(counts)])

        # slot arrays, (chunk, partition) order inside each window
        srcg = np.zeros((128, NWIN * CHW), dtype=np.int32)
        lcol = np.full((128, NWIN * CHW), -1.0, dtype=np.float32)
        lrow = np.full((1, ES), -1.0, dtype=np.float32)
        eat = np.zeros((ED, ES), dtype=np.float32)

        for w in range(NWIN):
            k = counts[w]
            sl = slice(starts[w], starts[w + 1])
            j = np.arange(k)
            ch = w * CHW + j // 128
            p = j % 128
            srcg[p, ch] = s_c[sl]
            lcol[p, ch] = l_c[sl]
            pos = ch * 128 + p
            lrow[0, pos] = l_c[sl]
            eat[:, pos] = ea_c[sl].T

        # host-built stacked stationary [S^T(112) ; ea(16)] per edge slot
        stk = np.zeros((128, ES), dtype=BF16)
        lr = lrow[0]
        pos_valid = np.nonzero(lr >= 0)[0]
        stk[lr[pos_valid].astype(np.int64), pos_valid] = 1.0
        stk[WN:] = eat.astype(BF16)

        per_core.append(
            dict(
                srcg=srcg,
                lcol=lcol.astype(BF16),
                lrow=lrow.astype(BF16),
                eat=eat.astype(BF16),
                stk=stk,
                invc=invc_full[c * NS:(c + 1) * NS].reshape(NWIN, WN).T.copy(),
            )
        )
    return per_core


def _prep_inputs(inputs):
    x = inputs["x"]
    xp = np.zeros((NP, D), dtype=np.float32)
    xp[:N] = x
    per_core_edges = _prep_edges(np.asarray(inputs["edge_index"]),
                                 np.asarray(inputs["edge_attr"]))

    iotar = np.broadcast_to(np.arange(WN, dtype=np.float32), (128, WN))
    iotac = np.arange(WN, dtype=np.float32).reshape(WN, 1)
    ones1 = np.ones((1, 128), dtype=np.float32)

    blocks = []
    for i in range(1, 5):
        We = np.asarray(inputs[f"We{i}"], np.float32)
        be = np.asarray(inputs[f"be{i}"], np.float32)
        Wn = np.asarray(inputs[f"Wn{i}"], np.float32)
        bn = np.asarray(inputs[f"bn{i}"], np.float32)
        din = We.shape[0] - ED
        din //= 2
        dout = We.shape[1]
        WeS, WeD, WeE = We[:din], We[din:2 * din], We[2 * din:]
        # pad dout -> 128
        wesd = np.zeros((128, 256), np.float32)
        wesd[:din, :dout] = WeS
        wesd[:din, 128:128 + dout] = WeD
        berow = np.zeros((1, 256), np.float32)
        berow[0, 128:128 + dout] = be
        wee = np.zeros((ED, 128), np.float32)
        wee[:, :dout] = WeE
        wnt = np.zeros((128, dout), np.float32)
        wnt[:din] = Wn[:din]
        wnb = np.zeros((128, dout), np.float32)
        wnb[:dout] = Wn[din:]
        bncol = bn.reshape(dout, 1).astype(np.float32)
        b = dict(wesd=wesd, berow=berow.astype(BF16),
                 wee=wee.astype(BF16), wnt=wnt,
                 wnb=wnb.astype(BF16), bn=bncol)
        if i == 4:
            # slim block-4 params: dout=1, keep only the real column
            wesd4b = np.zeros((128, 2), np.float32)
            wesd4b[:din, 0] = WeS[:, 0]
            wesd4b[:din, 1] = WeD[:, 0]
            berow4b = np.zeros((1, 2), np.float32)
            berow4b[0, 1] = be[0]
            b["wesdb"] = wesd4b
            b["berowb"] = berow4b.astype(BF16)
            b["weeb"] = np.tile(WeE[:, :1], (1, NWIN)).astype(BF16)
        blocks.append(b)

    in_maps = []
    for c in range(NCORES):
        m = dict(
            xT=xp[c * NS:(c + 1) * NS].T.copy(),
            srcg=per_core_edges[c]["srcg"],
            lcol=per_core_edges[c]["lcol"],
            lrow=per_core_edges[c]["lrow"],
            eat=per_core_edges[c]["eat"],
            stk=per_core_edges[c]["stk"],
            invc=per_core_edges[c]["invc"],
            iotar=iotar.astype(BF16),
            iotac=iotac.astype(np.float32),
            ones1=ones1.astype(BF16),
        )
        for i, b in enumerate(blocks, 1):
            for k, v in b.items():
                m[f"{k}{i}"] = v
        in_maps.append(m)
    return in_maps


# ---------------------------------------------------------------------------
# numpy emulation of the device dataflow (for fast correctness checking)
# ---------------------------------------------------------------------------

def _emulate(in_maps):
    import jax

    f32 = np.float32
    outs = []
    HT = [m["xT"].astype(f32).copy() for m in in_maps]
    for i in range(1, 5):
        Us, Vs = [], []
        for c, m in enumerate(in_maps):
            wesd = m[f"wesd{i}"].astype(f32)
            uv = HT[c].T @ wesd  # [NS, 256]
            uv += np.ones((NS, 1), f32) @ m[f"berow{i}"].astype(f32)
            Us.append(uv[:, :128].astype(BF16))
            Vs.append(uv[:, 128:].astype(BF16))
        U_full = np.concatenate(Us, 0)  # bf16 allgather
        for c, m in enumerate(in_maps):
            V = Vs[c].astype(f32)
            # slot s = g*128 + p  (g = global chunk)
            src_s = m["srcg"].T.reshape(-1)
            l_s = m["lcol"].T.reshape(-1).astype(f32)
            valid = l_s >= 0
            win_s = np.arange(ES) // (CHW * 128)
            dst_s = (win_s * WN + l_s.astype(np.int64).clip(0))
            u = U_full[src_s].astype(f32)
            v = np.where(valid[:, None], V[dst_s], 0.0)
            ew = m["eat"].T.astype(f32) @ m[f"wee{i}"].astype(f32)
            msg = np.maximum(u + v + ew, 0).astype(BF16).astype(f32)
            agg = np.array(jax.ops.segment_sum(
                msg[valid], dst_s[valid], num_segments=NS))
            agg *= m["invc"].T.reshape(-1)[:, None]
            aggT = agg.astype(BF16).astype(f32).T
            hT = m[f"wnt{i}"].astype(f32).T @ HT[c]
            hT += m[f"wnb{i}"].astype(f32).T @ aggT
            hT += m[f"bn{i}"].astype(f32)
            if i < 4:
                HT[c] = np.maximum(hT, 0)
            else:
                outs.append(1.0 / (1.0 + np.exp(-hT[0])))
    return np.concatenate(outs)[:N].reshape(N, 1).astype(np.float32)


# ---------------------------------------------------------------------------
# bass program
# ---------------------------------------------------------------------------

def _build():
    from concourse import bacc, bass, mybir, tile
    from concourse.masks import make_identity

    f32 = mybir.dt.float32
    bf16 = mybir.dt.bfloat16
    i32 = mybir.dt.int32

    nc = bacc.Bacc("TRN2", num_devices=NCORES)

    inp = {}
    for name, shape, dt in [
        ("xT", [128, NS], f32),
        ("srcg", [128, NWIN * CHW], i32),
        ("lcol", [128, NWIN * CHW], bf16),
        ("stk", [128, ES], bf16),
        ("invc", [WN, NWIN], f32),
        ("iotar", [128, WN], bf16),
        ("ones1", [1, 128], bf16),
    ]:
        inp[name] = nc.dram_tensor(name, shape, dt, kind="ExternalInput")
    for i in range(1, 5):
        dout = 1 if i == 4 else D
        for name, shape, dt in [
            (f"wesd{i}", [128, 256], f32),
            (f"berow{i}", [1, 256], bf16),
            (f"wee{i}", [ED, 128], bf16),
            (f"wnt{i}", [128, dout], f32),
            (f"wnb{i}", [128, dout], bf16),
            (f"bn{i}", [dout, 1], f32),
        ]:
            inp[name] = nc.dram_tensor(name, shape, dt, kind="ExternalInput")
    for name, shape, dt in [
        ("wesdb4", [128, 2], f32),
        ("berowb4", [1, 2], bf16),
        ("weeb4", [ED, NWIN], bf16),
    ]:
        inp[name] = nc.dram_tensor(name, shape, dt, kind="ExternalInput")
    out_ext = nc.dram_tensor("out", [1, NS], f32, kind="ExternalOutput")

    with tile.TileContext(nc) as tc:
        with tc.tile_pool(name="res", bufs=1) as res, \
             tc.tile_pool(name="sb", bufs=2) as sb, \
             tc.tile_pool(name="pbig", bufs=2, space="PSUM") as pbig, \
             tc.tile_pool(name="psmall", bufs=2, space="PSUM") as psmall, \
             tc.tile_pool(name="dram", bufs=1, space="DRAM") as dram:

            # resident tensors
            hT = res.tile([128, NS], f32)          # node features, transposed
            # [V_win(112) ; WeE(16)] per window, window w at cols w*128
            vw = res.tile([128, NWIN * 128], bf16)
            aggT = res.tile([128, NS], bf16)
            invc_sb = res.tile([WN, NWIN], f32)
            iotar_sb = res.tile([128, WN], bf16)
            ones_sb = res.tile([1, 128], bf16)
            ident = res.tile([128, 128], bf16)

            nc.sync.dma_start(hT[:], inp["xT"][:])
            nc.sync.dma_start(invc_sb[:], inp["invc"][:])
            nc.sync.dma_start(iotar_sb[:], inp["iotar"][:])
            nc.sync.dma_start(ones_sb[:], inp["ones1"][:])
            make_identity(nc, ident[:])

            u_bounce = dram.tile([NS, 128], bf16)

            u4_bounce = dram.tile([NS, 2], bf16, name="u4_bounce",
                                  tag="u4_bounce")
            for i in range(1, 5):
                dout = 1 if i == 4 else D
                slim = i == 4
                wnt = sb.tile([128, dout], f32, tag="wnt")
                wnb = sb.tile([128, dout], bf16, tag="wnb")
                bncol = sb.tile([dout, 1], f32, tag="bncol")
                nc.sync.dma_start(wnt[:], inp[f"wnt{i}"][:])
                nc.sync.dma_start(wnb[:], inp[f"wnb{i}"][:])
                nc.sync.dma_start(bncol[:], inp[f"bn{i}"][:])
                if not slim:
                    u_full = dram.tile([NP, 128], bf16, addr_space="Shared",
                                       name=f"u_full{i}", tag=f"u_full{i}")
                    wesd = sb.tile([128, 256], f32, tag="wesd")
                    berow = sb.tile([1, 256], bf16, tag="berow")
                    nc.sync.dma_start(wesd[:], inp[f"wesd{i}"][:])
                    nc.sync.dma_start(berow[:], inp[f"berow{i}"][:])
                    # WeE replicated into rows 112:128 of every window block
                    nc.sync.dma_start(
                        vw[WN:128, :].rearrange("p (w d) -> p w d", w=NWIN),
                        inp[f"wee{i}"][:, None, :]
                        .to_broadcast([ED, NWIN, 128]))
                else:
                    u_full = dram.tile([NP, 2], bf16, addr_space="Shared",
                                       name="u4_full", tag="u4_full")
                    wesd = sb.tile([128, 2], f32, tag="wesdb")
                    berow = sb.tile([1, 2], bf16, tag="berowb")
                    nc.sync.dma_start(wesd[:], inp["wesdb4"][:])
                    nc.sync.dma_start(berow[:], inp["berowb4"][:])
                    vw4 = res.tile([128, NWIN], bf16)
                    nc.sync.dma_start(vw4[WN:128, :],
                                      inp["weeb4"][:])

                # ---- node-side: U,V tiles (row-major) ----
                nuv = 2 if slim else 256
                for t in range(NWIN):
                    puv = pbig.tile([128, 1024], f32, tag="pbig")
                    nc.tensor.matmul(out=puv[:WN, :nuv],
                                     lhsT=hT[:, t * WN:(t + 1) * WN],
                                     rhs=wesd[:], start=True, stop=False)
                    nc.tensor.matmul(out=puv[:WN, :nuv],
                                     lhsT=ones_sb[:, :WN],
                                     rhs=berow[:], start=False, stop=True)
                    if not slim:
                        utile = sb.tile([WN, 128], bf16, tag="utile")
                        nc.scalar.copy(utile[:], puv[:WN, :128])
                        nc.vector.tensor_copy(vw[:WN, t * 128:(t + 1) * 128],
                                              puv[:WN, 128:256])
                        nc.scalar.dma_start(u_bounce[t * WN:(t + 1) * WN, :],
                                            utile[:])
                    else:
                        utile = sb.tile([WN, 2], bf16, tag="utile4")
                        nc.scalar.copy(utile[:], puv[:WN, :2])
                        nc.vector.tensor_copy(vw4[:WN, t:t + 1],
                                              puv[:WN, 1:2])
                        nc.scalar.dma_start(u4_bounce[t * WN:(t + 1) * WN, :],
                                            utile[:])

                nc.gpsimd.collective_compute(
                    "AllGather", mybir.AluOpType.bypass,
                    replica_groups=[list(range(NCORES))],
                    ins=[(u4_bounce if slim else u_bounce).opt()],
                    outs=[u_full.opt()],
                )

                # ---- edge phase ----
                for w in range(NWIN):
                    if w % WQ == 0:
                        srcg_sl = sb.tile([128, WQ * CHW], i32, tag="srcg")
                        nc.sync.dma_start(
                            srcg_sl[:],
                            inp["srcg"][:, w * CHW:(w + WQ) * CHW])
                        lcol_sl = sb.tile([128, WQ * CHW], bf16, tag="lcolw")
                        nc.sync.dma_start(
                            lcol_sl[:],
                            inp["lcol"][:, w * CHW:(w + WQ) * CHW])
                    w0 = (w % WQ) * CHW
                    uslab = sb.tile(
                        [128, CHW * 2] if slim else [128, CHW * 128],
                        bf16, tag="uslab4" if slim else "uslab", bufs=3)
                    nc.gpsimd.indirect_dma_start(
                        out=uslab[:],
                        out_offset=None,
                        in_=u_full[:],
                        in_offset=bass.IndirectOffsetOnAxis(
                            ap=srcg_sl[:, w0:w0 + CHW], axis=0),
                    )
                    # stacked stationary [S^T(112) ; ea(16)] for the window
                    stslab = sb.tile([128, CHW * 128], bf16, tag="stslab",
                                     bufs=3)
                    nc.scalar.dma_start(
                        stslab[:],
                        inp["stk"][:, w * CHW * 128:(w + 1) * CHW * 128])

                    pw = psmall.tile([128, 128], f32, tag="pw")
                    for g0 in range(0, CHW, GRP):
                        sslab = sb.tile([128, GRP * WN], bf16, tag="sslab")
                        lc3 = lcol_sl[:, w0 + g0:w0 + g0 + GRP]
                        nc.vector.tensor_tensor(
                            out=sslab[:].rearrange("p (c e) -> p c e", c=GRP),
                            in0=lc3[:, :, None].to_broadcast([128, GRP, WN]),
                            in1=iotar_sb[:, None, :].to_broadcast(
                                [128, GRP, WN]),
                            op=mybir.AluOpType.is_equal)

                        if not slim:
                            pe_ = pbig.tile([128, GRP * 128], f32, tag="pbig")
                            for h in range(0, GRP, 4):
                                nc.tensor.matmul(
                                    out=pe_[:, h * 128:(h + 4) * 128],
                                    lhsT=ident[:],
                                    rhs=uslab[:, (g0 + h) * 128:
                                              (g0 + h + 4) * 128],
                                    start=True, stop=False)
                            for c in range(g0, g0 + GRP):
                                r = (c - g0) * 128
                                nc.tensor.matmul(
                                    out=pe_[:, r:r + 128],
                                    lhsT=stslab[:, c * 128:(c + 1) * 128],
                                    rhs=vw[:, w * 128:(w + 1) * 128],
                                    start=False, stop=True)
                            wslab = sb.tile([128, GRP * 128], bf16,
                                            tag="wslab")
                            nc.scalar.activation(
                                wslab[:], pe_[:],
                                mybir.ActivationFunctionType.Relu)
                            for c in range(g0, g0 + GRP):
                                r = (c - g0) * 128
                                nc.tensor.matmul(
                                    out=pw[:WN, :],
                                    lhsT=sslab[:, r // 128 * WN:
                                               (r // 128 + 1) * WN],
                                    rhs=wslab[:, r:r + 128],
                                    start=(c == 0), stop=(c == CHW - 1))
                        else:
                            pe4 = pbig.tile([128, GRP], f32, tag="pbig")
                            for c in range(g0, g0 + GRP):
                                nc.tensor.matmul(
                                    out=pe4[:, c - g0:c - g0 + 1],
                                    lhsT=stslab[:, c * 128:(c + 1) * 128],
                                    rhs=vw4[:, w:w + 1],
                                    start=True, stop=True)
                            u4r = uslab[:].rearrange(
                                "p (c t) -> p c t", t=2)
                            nc.vector.tensor_tensor(
                                out=pe4[:], in0=pe4[:],
                                in1=u4r[:, g0:g0 + GRP, 0:1],
                                op=mybir.AluOpType.add)
                            wslab4 = sb.tile([128, GRP], bf16, tag="wslab4")
                            nc.scalar.activation(
                                wslab4[:], pe4[:],
                                mybir.ActivationFunctionType.Relu)
                            for c in range(g0, g0 + GRP):
                                cc = c - g0
                                nc.tensor.matmul(
                                    out=pw[:WN, :1],
                                    lhsT=sslab[:, cc * WN:(cc + 1) * WN],
                                    rhs=wslab4[:, cc:cc + 1],
                                    start=(c == 0), stop=(c == CHW - 1))
                    # scatter-mean + transpose into aggT
                    nd = 1 if slim else 128
                    argm = sb.tile([WN, nd], bf16,
                                   tag="argm4" if slim else "argm")
                    nc.vector.tensor_scalar(
                        out=argm[:], in0=pw[:WN, :nd],
                        scalar1=invc_sb[:, w:w + 1], scalar2=None,
                        op0=mybir.AluOpType.mult)
                    pt = psmall.tile([nd, WN], bf16, tag="pt")
                    nc.tensor.transpose(pt[:], argm[:], ident[:WN, :WN])
                    nc.scalar.copy(aggT[:nd, w * WN:(w + 1) * WN], pt[:])

                # ---- node update ----
                nsz = [512] * (NS // 512) + ([NS % 512] if NS % 512 else [])
                off = 0
                for sz in nsz:
                    ph = pbig.tile([128, 512], f32, tag="pbig")
                    nc.tensor.matmul(out=ph[:dout, :sz], lhsT=wnt[:],
                                     rhs=hT[:, off:off + sz],
                                     start=True, stop=False)
                    nc.tensor.matmul(out=ph[:dout, :sz], lhsT=wnb[:],
                                     rhs=aggT[:, off:off + sz],
                                     start=False, stop=True)
                    if i < 4:
                        nc.scalar.activation(
                            hT[:, off:off + sz], ph[:, :sz],
                            mybir.ActivationFunctionType.Relu,
                            bias=bncol[:])
                    else:
                        out_t = sb.tile([1, 512], f32, tag="out_t")
                        nc.scalar.activation(
                            out_t[:, :sz], ph[:dout, :sz],
                            mybir.ActivationFunctionType.Sigmoid,
                            bias=bncol[:])
                        nc.sync.dma_start(out_ext[:, off:off + sz],
                                          out_t[:, :sz])
                    off += sz

    nc.finalize()
    return nc


_NC_CACHE = {}


def kernel(**inputs):
    from concourse.bass_utils import run_bass_kernel_spmd

    in_maps = _prep_inputs(inputs)
    if "nc" not in _NC_CACHE:
        _NC_CACHE["nc"] = _build()
    nc = _NC_CACHE["nc"]
    res = run_bass_kernel_spmd(nc, in_maps, core_ids=list(range(NCORES)))
    outs = [res.results[c]["out"].reshape(-1) for c in range(NCORES)]
    return np.concatenate(outs)[:N].reshape(N, 1).astype(np.float32)


# revision 37
# speedup vs baseline: 3.1804x; 1.0043x over previous
"""Distributed Trainium2 kernel for the 4-block GNN (nn_ActorGNN).

Strategy (edge-parallel, dst-sharded):
  - Pad N=100000 -> NP=100352 = 8 * 12544 nodes; core c owns nodes
    [c*12544, (c+1)*12544).  Node features live transposed in SBUF (H^T).
  - Algebra: the edge MLP  relu([x_src|x_dst|ea] @ We + be)  is split as
    relu(U[src] + V[dst] + ea@WeE + be) with U = x@WeS, V = x@WeD computed
    per node shard (cheap N-side matmuls).
  - U is AllGathered (bf16); every core gathers arbitrary source rows with
    SWDGE indirect DMA; V/agg stay core-local (edges live on the core that
    owns their destination).
  - Edges are grouped by destination window of 112 nodes and padded to a
    uniform 32 chunks x 128 edges per window (SPMD-uniform).  112 was
    chosen so the expand matmul's contraction packs [S^T(112) ; ea(16)]
    against [V_win(112) ; WeE(16)] - the edge-attr matmul rides along for
    free.  Per chunk, PE does only two matmuls: the merged expand and the
    one-hot segment-reduce; gathered U rows are injected via one
    identity-matmul per 4 chunks.  scatter-mean = PSUM accumulate over the
    window + multiply by 1/max(cnt,1).
"""

import numpy as np
import ml_dtypes

BF16 = ml_dtypes.bfloat16

N = 100_000
E = 3_200_000
D = 128
ED = 16
NCORES = 8
NS = 12_544           # nodes per core
NP = NS * NCORES      # padded node count
WN = 112              # nodes per window (112 + 16 ea rows = 128 = PE K)
NWIN = NS // WN       # 112 windows per core
CHW = 32              # chunks (of 128 edges) per window, uniform
ES = NWIN * CHW * 128  # padded edge slots per core
GRP = 8               # chunks per relu/S-slab/u-inject group
WQ = 4                # windows per srcg/lcol slab load


# ---------------------------------------------------------------------------
# host-side preparation
# ---------------------------------------------------------------------------

def _prep_edges(edge_index, edge_attr):
    """Distribute edges to cores/windows; build per-core slot arrays."""
    src = edge_index[0].astype(np.int64)
    dst = edge_index[1].astype(np.int64)

    cnt = np.bincount(dst, minlength=NP).astype(np.float32)
    invc_full = 1.0 / np.maximum(cnt, 1.0)

    core = dst // NS
    win = (dst % NS) // WN
    l = dst % WN

    per_core = []
    for c in range(NCORES):
        m = core == c
        s_c, w_c, l_c = src[m], win[m], l[m]
        order = np.argsort(w_c, kind="stable")
        s_c, w_c, l_c = s_c[order], w_c[order], l_c[order]
        ea_c = edge_attr[m][order]

        counts = np.bincount(w_c, minlength=NWIN)
        assert counts.max() <= CHW * 128, f"window overflow: {counts.max()}"
        starts = np.concatenate([[0], np.cumsum(counts)])

        # slot arrays, (chunk, partition) order inside each window
        srcg = np.zeros((128, NWIN * CHW), dtype=np.int32)
        lcol = np.full((128, NWIN * CHW), -1.0, dtype=np.float32)
        lrow = np.full((1, ES), -1.0, dtype=np.float32)
        eat = np.zeros((ED, ES), dtype=np.float32)

        for w in range(NWIN):
            k = counts[w]
            sl = slice(starts[w], starts[w + 1])
            j = np.arange(k)
            ch = w * CHW + j // 128
            p = j % 128
            srcg[p, ch] = s_c[sl]
            lcol[p, ch] = l_c[sl]
            pos = ch * 128 + p
            lrow[0, pos] = l_c[sl]
            eat[:, pos] = ea_c[sl].T

        # host-built stacked stationary [S^T(112) ; ea(16)] per edge slot
        stk = np.zeros((128, ES), dtype=BF16)
        lr = lrow[0]
        pos_valid = np.nonzero(lr >= 0)[0]
        stk[lr[pos_valid].astype(np.int64), pos_valid] = 1.0
        stk[WN:] = eat.astype(BF16)

        per_core.append(
            dict(
                srcg=srcg,
                lcol=lcol.astype(BF16),
                lrow=lrow.astype(BF16),
                eat=eat.astype(BF16),
                stk=stk,
                invc=invc_full[c * NS:(c + 1) * NS].reshape(NWIN, WN).T.copy(),
            )
        )
    return per_core


def _prep_inputs(inputs):
    x = inputs["x"]
    xp = np.zeros((NP, D), dtype=np.float32)
    xp[:N] = x
    per_core_edges = _prep_edges(np.asarray(inputs["edge_index"]),
                                 np.asarray(inputs["edge_attr"]))

    iotar = np.broadcast_to(np.arange(WN, dtype=np.float32), (128, WN))
    iotac = np.arange(WN, dtype=np.float32).reshape(WN, 1)
    ones1 = np.ones((1, 128), dtype=np.float32)

    blocks = []
    for i in range(1, 5):
        We = np.asarray(inputs[f"We{i}"], np.float32)
        be = np.asarray(inputs[f"be{i}"], np.float32)
        Wn = np.asarray(inputs[f"Wn{i}"], np.float32)
        bn = np.asarray(inputs[f"bn{i}"], np.float32)
        din = We.shape[0] - ED
        din //= 2
        dout = We.shape[1]
        WeS, WeD, WeE = We[:din], We[din:2 * din], We[2 * din:]
        # pad dout -> 128
        wesd = np.zeros((128, 256), np.float32)
        wesd[:din, :dout] = WeS
        wesd[:din, 128:128 + dout] = WeD
        berow = np.zeros((1, 256), np.float32)
        berow[0, 128:128 + dout] = be
        wee = np.zeros((ED, 128), np.float32)
        wee[:, :dout] = WeE
        wnt = np.zeros((128, dout), np.float32)
        wnt[:din] = Wn[:din]
        wnb = np.zeros((128, dout), np.float32)
        wnb[:dout] = Wn[din:]
        bncol = bn.reshape(dout, 1).astype(np.float32)
        b = dict(wesd=wesd, berow=berow.astype(BF16),
                 wee=wee.astype(BF16), wnt=wnt,
                 wnb=wnb.astype(BF16), bn=bncol)
        if i == 4:
            # slim block-4 params: dout=1, keep only the real column
            wesd4b = np.zeros((128, 2), np.float32)
            wesd4b[:din, 0] = WeS[:, 0]
            wesd4b[:din, 1] = WeD[:, 0]
            berow4b = np.zeros((1, 2), np.float32)
            berow4b[0, 1] = be[0]
            b["wesdb"] = wesd4b
            b["berowb"] = berow4b.astype(BF16)
            b["weeb"] = np.tile(WeE[:, :1], (1, NWIN)).astype(BF16)
        blocks.append(b)

    in_maps = []
    for c in range(NCORES):
        m = dict(
            xT=xp[c * NS:(c + 1) * NS].T.copy(),
            srcg=per_core_edges[c]["srcg"],
            lcol=per_core_edges[c]["lcol"],
            lrow=per_core_edges[c]["lrow"],
            eat=per_core_edges[c]["eat"],
            stk=per_core_edges[c]["stk"],
            invc=per_core_edges[c]["invc"],
            iotar=iotar.astype(BF16),
            iotac=iotac.astype(np.float32),
            ones1=ones1.astype(BF16),
        )
        for i, b in enumerate(blocks, 1):
            for k, v in b.items():
                m[f"{k}{i}"] = v
        in_maps.append(m)
    return in_maps


# ---------------------------------------------------------------------------
# numpy emulation of the device dataflow (for fast correctness checking)
# ---------------------------------------------------------------------------

def _emulate(in_maps):
    import jax

    f32 = np.float32
    outs = []
    HT = [m["xT"].astype(f32).copy() for m in in_maps]
    for i in range(1, 5):
        Us, Vs = [], []
        for c, m in enumerate(in_maps):
            wesd = m[f"wesd{i}"].astype(f32)
            uv = HT[c].T @ wesd  # [NS, 256]
            uv += np.ones((NS, 1), f32) @ m[f"berow{i}"].astype(f32)
            Us.append(uv[:, :128].astype(BF16))
            Vs.append(uv[:, 128:].astype(BF16))
        U_full = np.concatenate(Us, 0)  # bf16 allgather
        for c, m in enumerate(in_maps):
            V = Vs[c].astype(f32)
            # slot s = g*128 + p  (g = global chunk)
            src_s = m["srcg"].T.reshape(-1)
            l_s = m["lcol"].T.reshape(-1).astype(f32)
            valid = l_s >= 0
            win_s = np.arange(ES) // (CHW * 128)
            dst_s = (win_s * WN + l_s.astype(np.int64).clip(0))
            u = U_full[src_s].astype(f32)
            v = np.where(valid[:, None], V[dst_s], 0.0)
            ew = m["eat"].T.astype(f32) @ m[f"wee{i}"].astype(f32)
            msg = np.maximum(u + v + ew, 0).astype(BF16).astype(f32)
            agg = np.array(jax.ops.segment_sum(
                msg[valid], dst_s[valid], num_segments=NS))
            agg *= m["invc"].T.reshape(-1)[:, None]
            aggT = agg.astype(BF16).astype(f32).T
            hT = m[f"wnt{i}"].astype(f32).T @ HT[c]
            hT += m[f"wnb{i}"].astype(f32).T @ aggT
            hT += m[f"bn{i}"].astype(f32)
            if i < 4:
                HT[c] = np.maximum(hT, 0)
            else:
                outs.append(1.0 / (1.0 + np.exp(-hT[0])))
    return np.concatenate(outs)[:N].reshape(N, 1).astype(np.float32)


# ---------------------------------------------------------------------------
# bass program
# ---------------------------------------------------------------------------

def _build():
    from concourse import bacc, bass, mybir, tile
    from concourse.masks import make_identity

    f32 = mybir.dt.float32
    bf16 = mybir.dt.bfloat16
    i32 = mybir.dt.int32

    nc = bacc.Bacc("TRN2", num_devices=NCORES)

    inp = {}
    for name, shape, dt in [
        ("xT", [128, NS], f32),
        ("srcg", [128, NWIN * CHW], i32),
        ("lcol", [128, NWIN * CHW], bf16),
        ("stk", [128, ES], bf16),
        ("invc", [WN, NWIN], f32),
        ("iotar", [128, WN], bf16),
        ("ones1", [1, 128], bf16),
    ]:
        inp[name] = nc.dram_tensor(name, shape, dt, kind="ExternalInput")
    for i in range(1, 5):
        dout = 1 if i == 4 else D
        for name, shape, dt in [
            (f"wesd{i}", [128, 256], f32),
            (f"berow{i}", [1, 256], bf16),
            (f"wee{i}", [ED, 128], bf16),
            (f"wnt{i}", [128, dout], f32),
            (f"wnb{i}", [128, dout], bf16),
            (f"bn{i}", [dout, 1], f32),
        ]:
            inp[name] = nc.dram_tensor(name, shape, dt, kind="ExternalInput")
    for name, shape, dt in [
        ("wesdb4", [128, 2], f32),
        ("berowb4", [1, 2], bf16),
        ("weeb4", [ED, NWIN], bf16),
    ]:
        inp[name] = nc.dram_tensor(name, shape, dt, kind="ExternalInput")
    out_ext = nc.dram_tensor("out", [1, NS], f32, kind="ExternalOutput")

    with tile.TileContext(nc) as tc:
        with tc.tile_pool(name="res", bufs=1) as res, \
             tc.tile_pool(name="sb", bufs=2) as sb, \
             tc.tile_pool(name="pbig", bufs=2, space="PSUM") as pbig, \
             tc.tile_pool(name="psmall", bufs=2, space="PSUM") as psmall, \
             tc.tile_pool(name="dram", bufs=1, space="DRAM") as dram:

            # resident tensors
            hT = res.tile([128, NS], f32)          # node features, transposed
            # [V_win(112) ; WeE(16)] per window, window w at cols w*128
            vw = res.tile([128, NWIN * 128], bf16)
            aggT = res.tile([128, NS], bf16)
            invc_sb = res.tile([WN, NWIN], f32)
            iotar_sb = res.tile([128, WN], bf16)
            ones_sb = res.tile([1, 128], bf16)
            ident = res.tile([128, 128], bf16)

            nc.sync.dma_start(hT[:], inp["xT"][:])
            nc.sync.dma_start(invc_sb[:], inp["invc"][:])
            nc.sync.dma_start(iotar_sb[:], inp["iotar"][:])
            nc.sync.dma_start(ones_sb[:], inp["ones1"][:])
            make_identity(nc, ident[:])

            u_bounce = dram.tile([NS, 128], bf16)

            u4_bounce = dram.tile([NS, 2], bf16, name="u4_bounce",
                                  tag="u4_bounce")
            for i in range(1, 5):
                dout = 1 if i == 4 else D
                slim = i == 4
                wnt = sb.tile([128, dout], f32, tag="wnt")
                wnb = sb.tile([128, dout], bf16, tag="wnb")
                bncol = sb.tile([dout, 1], f32, tag="bncol")
                nc.sync.dma_start(wnt[:], inp[f"wnt{i}"][:])
                nc.sync.dma_start(wnb[:], inp[f"wnb{i}"][:])
                nc.sync.dma_start(bncol[:], inp[f"bn{i}"][:])
                if not slim:
                    u_full = dram.tile([NP, 128], bf16, addr_space="Shared",
                                       name=f"u_full{i}", tag=f"u_full{i}")
                    wesd = sb.tile([128, 256], f32, tag="wesd")
                    berow = sb.tile([1, 256], bf16, tag="berow")
                    nc.sync.dma_start(wesd[:], inp[f"wesd{i}"][:])
                    nc.sync.dma_start(berow[:], inp[f"berow{i}"][:])
                    # WeE replicated into rows 112:128 of every window block
                    nc.sync.dma_start(
                        vw[WN:128, :].rearrange("p (w d) -> p w d", w=NWIN),
                        inp[f"wee{i}"][:, None, :]
                        .to_broadcast([ED, NWIN, 128]))
                else:
                    u_full = dram.tile([NP, 2], bf16, addr_space="Shared",
                                       name="u4_full", tag="u4_full")
                    wesd = sb.tile([128, 2], f32, tag="wesdb")
                    berow = sb.tile([1, 2], bf16, tag="berowb")
                    nc.sync.dma_start(wesd[:], inp["wesdb4"][:])
                    nc.sync.dma_start(berow[:], inp["berowb4"][:])
                    vw4 = res.tile([128, NWIN], bf16)
                    nc.sync.dma_start(vw4[WN:128, :],
                                      inp["weeb4"][:])

                # ---- node-side: U,V tiles (row-major) ----
                nuv = 2 if slim else 256
                for t in range(NWIN):
                    puv = pbig.tile([128, 1024], f32, tag="pbig")
                    nc.tensor.matmul(out=puv[:WN, :nuv],
                                     lhsT=hT[:, t * WN:(t + 1) * WN],
                                     rhs=wesd[:], start=True, stop=False)
                    nc.tensor.matmul(out=puv[:WN, :nuv],
                                     lhsT=ones_sb[:, :WN],
                                     rhs=berow[:], start=False, stop=True)
                    if not slim:
                        utile = sb.tile([WN, 128], bf16, tag="utile")
                        nc.scalar.copy(utile[:], puv[:WN, :128])
                        nc.vector.tensor_copy(vw[:WN, t * 128:(t + 1) * 128],
                                              puv[:WN, 128:256])
                        nc.scalar.dma_start(u_bounce[t * WN:(t + 1) * WN, :],
                                            utile[:])
                    else:
                        utile = sb.tile([WN, 2], bf16, tag="utile4")
                        nc.scalar.copy(utile[:], puv[:WN, :2])
                        nc.vector.tensor_copy(vw4[:WN, t:t + 1],
                                              puv[:WN, 1:2])
                        nc.scalar.dma_start(u4_bounce[t * WN:(t + 1) * WN, :],
                                            utile[:])

                nc.gpsimd.collective_compute(
                    "AllGather", mybir.AluOpType.bypass,
                    replica_groups=[list(range(NCORES))],
                    ins=[(u4_bounce if slim else u_bounce).opt()],
                    outs=[u_full.opt()],
                )

                # ---- edge phase ----
                for w in range(NWIN):
                    if w % WQ == 0:
                        srcg_sl = sb.tile([128, WQ * CHW], i32, tag="srcg")
                        nc.sync.dma_start(
                            srcg_sl[:],
                            inp["srcg"][:, w * CHW:(w + WQ) * CHW])
                        lcol_sl = sb.tile([128, WQ * CHW], bf16, tag="lcolw")
                        nc.sync.dma_start(
                            lcol_sl[:],
                            inp["lcol"][:, w * CHW:(w + WQ) * CHW])
                    w0 = (w % WQ) * CHW
                    uslab = sb.tile(
                        [128, CHW * 2] if slim else [128, CHW * 128],
                        bf16, tag="uslab4" if slim else "uslab", bufs=3)
                    nc.gpsimd.indirect_dma_start(
                        out=uslab[:],
                        out_offset=None,
                        in_=u_full[:],
                        in_offset=bass.IndirectOffsetOnAxis(
                            ap=srcg_sl[:, w0:w0 + CHW], axis=0),
                    )
                    # stacked stationary [S^T(112) ; ea(16)] for the window
                    stslab = sb.tile([128, CHW * 128], bf16, tag="stslab",
                                     bufs=3)
                    nc.scalar.dma_start(
                        stslab[:],
                        inp["stk"][:, w * CHW * 128:(w + 1) * CHW * 128])

                    pw = psmall.tile([128, 128], f32, tag="pw")
                    for g0 in range(0, CHW, GRP):
                        sslab = sb.tile([128, GRP * WN], bf16, tag="sslab", bufs=3)
                        lc3 = lcol_sl[:, w0 + g0:w0 + g0 + GRP]
                        nc.vector.tensor_tensor(
                            out=sslab[:].rearrange("p (c e) -> p c e", c=GRP),
                            in0=lc3[:, :, None].to_broadcast([128, GRP, WN]),
                            in1=iotar_sb[:, None, :].to_broadcast(
                                [128, GRP, WN]),
                            op=mybir.AluOpType.is_equal)

                        if not slim:
                            pe_ = pbig.tile([128, GRP * 128], f32, tag="pbig")
                            for h in range(0, GRP, 4):
                                nc.tensor.matmul(
                                    out=pe_[:, h * 128:(h + 4) * 128],
                                    lhsT=ident[:],
                                    rhs=uslab[:, (g0 + h) * 128:
                                              (g0 + h + 4) * 128],
                                    start=True, stop=False)
                            for c in range(g0, g0 + GRP):
                                r = (c - g0) * 128
                                nc.tensor.matmul(
                                    out=pe_[:, r:r + 128],
                                    lhsT=stslab[:, c * 128:(c + 1) * 128],
                                    rhs=vw[:, w * 128:(w + 1) * 128],
                                    start=False, stop=True)
                            wslab = sb.tile([128, GRP * 128], bf16,
                                            tag="wslab", bufs=3)
                            nc.scalar.activation(
                                wslab[:], pe_[:],
                                mybir.ActivationFunctionType.Relu)
                            for c in range(g0, g0 + GRP):
                                r = (c - g0) * 128
                                nc.tensor.matmul(
                                    out=pw[:WN, :],
                                    lhsT=sslab[:, r // 128 * WN:
                                               (r // 128 + 1) * WN],
                                    rhs=wslab[:, r:r + 128],
                                    start=(c == 0), stop=(c == CHW - 1))
                        else:
                            pe4 = pbig.tile([128, GRP], f32, tag="pbig")
                            for c in range(g0, g0 + GRP):
                                nc.tensor.matmul(
                                    out=pe4[:, c - g0:c - g0 + 1],
                                    lhsT=stslab[:, c * 128:(c + 1) * 128],
                                    rhs=vw4[:, w:w + 1],
                                    start=True, stop=True)
                            u4r = uslab[:].rearrange(
                                "p (c t) -> p c t", t=2)
                            nc.vector.tensor_tensor(
                                out=pe4[:], in0=pe4[:],
                                in1=u4r[:, g0:g0 + GRP, 0:1],
                                op=mybir.AluOpType.add)
                            wslab4 = sb.tile([128, GRP], bf16, tag="wslab4")
                            nc.scalar.activation(
                                wslab4[:], pe4[:],
                                mybir.ActivationFunctionType.Relu)
                            for c in range(g0, g0 + GRP):
                                cc = c - g0
                                nc.tensor.matmul(
                                    out=pw[:WN, :1],
                                    lhsT=sslab[:, cc * WN:(cc + 1) * WN],
                                    rhs=wslab4[:, cc:cc + 1],
                                    start=(c == 0), stop=(c == CHW - 1))
                    # scatter-mean + transpose into aggT
                    nd = 1 if slim else 128
                    argm = sb.tile([WN, nd], bf16,
                                   tag="argm4" if slim else "argm")
                    nc.vector.tensor_scalar(
                        out=argm[:], in0=pw[:WN, :nd],
                        scalar1=invc_sb[:, w:w + 1], scalar2=None,
                        op0=mybir.AluOpType.mult)
                    pt = psmall.tile([nd, WN], bf16, tag="pt")
                    nc.tensor.transpose(pt[:], argm[:], ident[:WN, :WN])
                    nc.scalar.copy(aggT[:nd, w * WN:(w + 1) * WN], pt[:])

                # ---- node update ----
                nsz = [512] * (NS // 512) + ([NS % 512] if NS % 512 else [])
                off = 0
                for sz in nsz:
                    ph = pbig.tile([128, 512], f32, tag="pbig")
                    nc.tensor.matmul(out=ph[:dout, :sz], lhsT=wnt[:],
                                     rhs=hT[:, off:off + sz],
                                     start=True, stop=False)
                    nc.tensor.matmul(out=ph[:dout, :sz], lhsT=wnb[:],
                                     rhs=aggT[:, off:off + sz],
                                     start=False, stop=True)
                    if i < 4:
                        nc.scalar.activation(
                            hT[:, off:off + sz], ph[:, :sz],
                            mybir.ActivationFunctionType.Relu,
                            bias=bncol[:])
                    else:
                        out_t = sb.tile([1, 512], f32, tag="out_t")
                        nc.scalar.activation(
                            out_t[:, :sz], ph[:dout, :sz],
                            mybir.ActivationFunctionType.Sigmoid,
                            bias=bncol[:])
                        nc.sync.dma_start(out_ext[:, off:off + sz],
                                          out_t[:, :sz])
                    off += sz

    nc.finalize()
    return nc


_NC_CACHE = {}


def kernel(**inputs):
    from concourse.bass_utils import run_bass_kernel_spmd

    in_maps = _prep_inputs(inputs)
    if "nc" not in _NC_CACHE:
        _NC_CACHE["nc"] = _build()
    nc = _NC_CACHE["nc"]
    res = run_bass_kernel_spmd(nc, in_maps, core_ids=list(range(NCORES)))
    outs = [res.results[c]["out"].reshape(-1) for c in range(NCORES)]
    return np.concatenate(outs)[:N].reshape(N, 1).astype(np.float32)
